# revision 1
# baseline (speedup 1.0000x reference)
"""CapsNet DigitCaps routing kernel for 8 TRN2 NeuronCores.

Strategy: shard the 1152 primary capsules across the 8 cores (144 each),
keep the full batch (256) on every core. Each core builds its slice of
u_hat = einsum('dpij,bpj->bdpi') once on the TensorEngine (block-diagonal
weight trick: 3 primary capsules per matmul, K=32 with zero padding,
4-way tile_position row-tiling, N=480 = one PSUM bank) and keeps it
resident in SBUF as bf16 (11.8MB).

Routing iterations (vs the earlier all-DVE version):
  s-phase (r=1,2):  Y = c*u_hat broadcast-multiply + pairwise add tree,
    split ~78/22 between DVE and the Pool engine (Pool is 4x slower per
    element but runs in parallel).
  g-phase (r=0,1):  computed in j-space instead of i-space:
       g[b,d,p] = sum_j x[b,p,j] * wv[b,d,p,j],
       wv[b,d,p,j] = sum_i W[d,p,i,j] * s[b,d,i]
    wv is a TensorEngine matmul (k=i=16, lhsT = s^T via PE transpose,
    rhs = per-digit W slices [16, j*p]); the alpha squash scale is folded
    into the PSUM->SBUF drain (scale-multiply). DVE then only does a
    half-size multiply (J=8 vs I=16) plus a 3-level j-tree.
  Build PSUM drains are spread across DVE/ACT/Pool.

The reference's _squash uses a GLOBAL Frobenius norm over the whole
[B,D,1,16] s tensor, which couples all batch elements and hence all
shards: each iteration does one small (164KB) AllReduce of the per-core
partial s, after which every core computes the identical squash scale
alpha = n2 / ((n2+1) * (sqrt(n2)+eps)) and proceeds identically.

Layouts (per core, SBUF partition dim first):
  u_hat  [128, 2*48*480] bf16   row=b%128, col = bt*23040 + g*480 + ph*160 + i*10 + d
                                 (bt=b//128, p_local = 3*g+ph)
  L      [128, 2*1440] f32      col = bt*1440 + d*144 + p   (d-major!)
  cpd    [128, 2*1440] bf16     col = bt*1440 + p*10 + d    (p-major, d innermost)
  wv     [128, 11520]  bf16     col = d*1152 + j*144 + p    (per-bt scratch)
  s,sv   [128, 2*160]  f32      col = bt*160 + i*10 + d
"""

import os
import sys

for _p in ("/opt/trn_rl_repo",):
    if _p not in sys.path and os.path.isdir(_p):
        sys.path.insert(0, _p)

import numpy as np
import ml_dtypes

import concourse.bass as bass
import concourse.bacc as bacc
import concourse.mybir as mybir
import concourse.tile as tile
from concourse.bass_utils import run_bass_kernel_spmd

F32 = mybir.dt.float32
BF16 = mybir.dt.bfloat16
MULT = mybir.AluOpType.mult
ADD = mybir.AluOpType.add
AF = mybir.ActivationFunctionType

B, D, P, I, J = 256, 10, 1152, 16, 8
CORES = 8
PL = P // CORES          # 144 local primary capsules
G = PL // 3              # 48 groups of 3 (block-diag build, N=480 = one PSUM bank)
GW = 3 * I * D           # 480 cols per group
NQ = G // 4              # 12 q-tiles of 4 groups stacked on 128 partitions
NKT = (PL * J) // 128    # 9 k-tiles of 128 (p,j)-rows
EPS = 1e-7
NROUT = 3

# Pool-engine share of the big elementwise ops (in 16-col blocks of 144).
POOL_BLKS = int(os.environ.get("BASSCAPS_POOL_BLKS", "24"))
# Pool-engine share of the g-phase (in digits of 10).
POOL_D = int(os.environ.get("BASSCAPS_POOL_D", "2"))
# Debug bisection: 0=build+s0, 1=+AllReduce, 2=+alpha/sv, 3=+g/L/softmax, 4=full
STAGE = int(os.environ.get("BASSCAPS_STAGE", "4"))
# Replace the AllReduce with a local DRAM copy (for TimelineSim profiling).
NO_CC = os.environ.get("BASSCAPS_NO_CC", "0") == "1"


def _pair_reduce(nc_engine, view_fn, nblk, blk_cols, add_op):
    """Pairwise-sum blocks [0, nblk) into block 0 via repeated halving.

    view_fn(lo, n) must return an AP covering blocks [lo, lo+n).
    Handles odd sizes by folding the tail block in at the next level.
    """
    carry = []
    n = nblk
    while n > 1:
        h = n // 2
        nc_engine.tensor_tensor(view_fn(0, h), view_fn(0, h), view_fn(h, h), add_op)
        if n % 2 == 1:
            carry.append(2 * h)
        n = h
    for cblk in carry:
        nc_engine.tensor_tensor(view_fn(0, 1), view_fn(0, 1), view_fn(cblk, 1), add_op)


def build_program():
    nc = bacc.Bacc("TRN2", target_bir_lowering=False, debug=False,
                   num_devices=CORES)

    # Per-core DRAM inputs (host pre-arranged, bf16).
    xl_d = nc.dram_tensor("x_l", [128, NQ * 256], BF16, kind="ExternalInput")
    wbd_d = nc.dram_tensor("w_bd", [128, NQ * GW], BF16, kind="ExternalInput")
    xpj_d = nc.dram_tensor("x_pj", [128, NKT * 256], BF16, kind="ExternalInput")
    wfl_d = nc.dram_tensor("w_fl", [128, NKT * 160], BF16, kind="ExternalInput")
    wg_d = nc.dram_tensor("w_g", [16, D * J * PL], BF16, kind="ExternalInput")
    xbp_d = nc.dram_tensor("x_bp", [128, 2 * J * PL], BF16, kind="ExternalInput")
    ident_d = nc.dram_tensor("ident", [128, 128], BF16, kind="ExternalInput")
    v_d = nc.dram_tensor("v_out", [B, D, I], F32, kind="ExternalOutput")

    DVE_BLKS = PL - POOL_BLKS           # 116 blocks of 16 cols for DVE
    DVE_D = D - POOL_D                  # 8 digits for DVE in g-phase

    with tile.TileContext(nc) as tc:
        with (
            tc.tile_pool(name="persist", bufs=1) as pp,
            tc.tile_pool(name="build", bufs=3) as bp,
            tc.tile_pool(name="psB", bufs=1, space=bass.MemorySpace.PSUM) as psB,
            tc.tile_pool(name="dram", bufs=1, space=bass.MemorySpace.DRAM) as dp,
        ):
            uh = pp.tile([128, 2 * G * GW], BF16)
            Lb = pp.tile([128, 2 * 1440], F32)
            cpd = pp.tile([128, 2 * 1440], BF16)
            expL = pp.tile([128, 1440], BF16)
            s_sb = pp.tile([128, 2 * 160], F32)
            # s^T staging: col = bt*320 + d*32 + i (16 pad cols per digit so
            # each digit's transposed rows start 32-aligned in PSUM)
            s_bf = pp.tile([128, 2 * 320], BF16)
            sv = pp.tile([128, 2 * 160], F32)
            Y = pp.tile([128, D * J * PL], BF16)   # s/g-phase scratch (11520)
            wv = Y           # g-phase scratch aliases Y (disjoint in time)
            wg = pp.tile([16, D * J * PL], BF16)
            xbp = pp.tile([128, 2 * J * PL], BF16)
            ident = pp.tile([128, 128], BF16)
            # per-digit s^T tiles [16(i), 2*128(bt,b)] — each at base
            # partition 0 (PE lhsT requires base partition 0/32/64)
            vtd = [pp.tile([16, 2 * 128], BF16, name=f"vt{d}")
                   for d in range(D)]
            zs = pp.tile([128, 5 * 144], BF16)
            zrec = pp.tile([128, 144], F32)
            acc = pp.tile([128, 1], F32)
            ones = pp.tile([128, 128], F32)
            ttr_junk = pp.tile([128, 320], F32)
            n2sb = pp.tile([128, 1], F32)
            t1 = pp.tile([128, 1], F32)
            r1 = pp.tile([128, 1], F32)
            lnv = pp.tile([128, 1], F32)
            rt = pp.tile([128, 1], F32)
            t2 = pp.tile([128, 1], F32)
            r2 = pp.tile([128, 1], F32)
            alpha_bc = pp.tile([128, 1], F32)

            bounce_in = dp.tile([B, 160], F32)
            bounce_out = dp.tile([B, 160], F32)

            nc.vector.memset(ones[:, :], 1.0)
            nc.gpsimd.memset(Lb[:, :], 0.0)
            nc.vector.memset(s_bf[:, :], 0.0)
            # warm the ACT exp/ln table set during the build
            nc.scalar.activation(t1[:, :], ones[:, :1], AF.Exp)
            nc.scalar.activation(t2[:, :], ones[:, :1], AF.Ln)

            # ---- load the small operands ----
            xpj = bp.tile([128, NKT * 256], BF16, tag="xpj")
            wfl = bp.tile([128, NKT * 160], BF16, tag="wfl")
            nc.sync.dma_start(xpj[:, :], xpj_d.ap())
            nc.sync.dma_start(wfl[:, :], wfl_d.ap())
            nc.sync.dma_start(wg[:, :], wg_d.ap())
            nc.sync.dma_start(xbp[:, :], xbp_d.ap())
            nc.sync.dma_start(ident[:, :], ident_d.ap())

            # ---- s0 = 0.1 * sum_p u_hat  (direct from x, W) ----
            for bt in range(2):
                ps0 = psB.tile([128, 160], F32, tag="s0")
                for kt in range(NKT):
                    nc.tensor.matmul(
                        ps0[:, :],
                        xpj[:, kt * 256 + bt * 128: kt * 256 + bt * 128 + 128],
                        wfl[:, kt * 160:(kt + 1) * 160],
                        start=(kt == 0), stop=(kt == NKT - 1),
                    )
                nc.scalar.mul(s_sb[:, bt * 160:(bt + 1) * 160], ps0[:, :], 0.1)

            # ---- build u_hat: 4-way row-tiled block-diag matmuls ----
            # group g = 3 primary capsules; lhsT = x block [32, 128]
            # (rows = (ph, j), ph==3 padding), rhs = W block-diag [32, 480].
            # Drains cycle DVE/ACT/Pool (3/3/2 per q-tile).
            with tc.tile_pool(name="psA", bufs=4, space=bass.MemorySpace.PSUM) as psA:
                drain_cycle = [0, 1, 0, 1, 0, 1, 0, 1]
                for q in range(NQ):
                    xlq = bp.tile([128, 256], BF16, tag="xl")
                    wbq = bp.tile([128, GW], BF16, tag="wb")
                    nc.sync.dma_start(xlq[:, :], xl_d.ap()[:, q * 256:(q + 1) * 256])
                    nc.sync.dma_start(wbq[:, :], wbd_d.ap()[:, q * GW:(q + 1) * GW])
                    for bt in range(2):
                        for gg in range(4):
                            g = q * 4 + gg
                            ps = psA.tile([128, GW], F32, tag="bld")
                            nc.tensor.matmul(
                                ps[:, :],
                                xlq[32 * gg:32 * (gg + 1),
                                    bt * 128:(bt + 1) * 128],
                                wbq[32 * gg:32 * (gg + 1), :],
                                start=True, stop=True,
                                tile_position=(32 * gg, 0),
                            )
                            dst = uh[:, bt * (G * GW) + g * GW:
                                     bt * (G * GW) + (g + 1) * GW]
                            eng = drain_cycle[(bt * 4 + gg) % 8]
                            if eng == 0:
                                nc.vector.tensor_copy(dst, ps[:, :])
                            elif eng == 1:
                                nc.scalar.copy(dst, ps[:, :])
                            else:
                                nc.gpsimd.tensor_copy(dst, ps[:, :])

            # wv PSUM pools open after the build pool closes (bank budget).
            with (
                tc.tile_pool(name="psW", bufs=2,
                             space=bass.MemorySpace.PSUM) as psW,
                tc.tile_pool(name="psT", bufs=1,
                             space=bass.MemorySpace.PSUM) as psT,
            ):

                def uh_ap(bt, lo, n):
                    """u_hat view: blocks of 16 cols -> (blk, i, d)."""
                    return uh[:, bt * (G * GW) + lo * 160:
                              bt * (G * GW) + (lo + n) * 160].rearrange(
                        "p (k i d) -> p k i d", k=n, i=16, d=10)

                def y_ap(lo, n):
                    return Y[:, lo * 160:(lo + n) * 160].rearrange(
                        "p (k i d) -> p k i d", k=n, i=16, d=10)

                for r in range(NROUT if STAGE >= 4 else 1):
                    if r > 0:
                        # ---- s_partial = sum_p c * u_hat ----
                        # two 72-block chunks per bt (Y holds 72 blocks)
                        CH = 72
                        DVE_C = CH - POOL_BLKS // 2      # 58
                        POOL_C = POOL_BLKS // 2          # 14
                        for bt in range(2):
                            cb = cpd[:, bt * 1440:(bt + 1) * 1440].rearrange(
                                "p (pp d) -> p pp d", pp=144, d=10)

                            def cview(lo, n):
                                return cb[:, lo:lo + n, None, :].to_broadcast(
                                    (128, n, 16, 10))

                            ss = s_sb[:, bt * 160:(bt + 1) * 160]
                            for ci in range(2):
                                clo = ci * CH
                                nc.vector.tensor_tensor(
                                    y_ap(0, DVE_C), uh_ap(bt, clo, DVE_C),
                                    cview(clo, DVE_C), MULT)
                                nc.gpsimd.tensor_tensor(
                                    y_ap(DVE_C, POOL_C),
                                    uh_ap(bt, clo + DVE_C, POOL_C),
                                    cview(clo + DVE_C, POOL_C), MULT)

                                yb = Y[:, :CH * 160].rearrange(
                                    "p (k x) -> p k x", k=CH, x=160)

                                def yview_d(lo, n):
                                    return yb[:, lo:lo + n, :]

                                def yview_p(lo, n):
                                    return yb[:, DVE_C + lo:DVE_C + lo + n, :]

                                _pair_reduce(nc.vector, yview_d, DVE_C, 160, ADD)
                                _pair_reduce(nc.gpsimd, yview_p, POOL_C, 160, ADD)
                                nc.vector.tensor_tensor(
                                    yb[:, 0, :], yb[:, 0, :], yb[:, DVE_C, :],
                                    ADD)
                                if ci == 0:
                                    nc.vector.tensor_copy(ss, yb[:, 0, :])
                                else:
                                    nc.vector.tensor_tensor(
                                        ss, ss, yb[:, 0, :], ADD)
                            if bt == 0:
                                bounce_bt(0)

                    # ---- AllReduce partial s over the 8 cores ----
                    # (per-bt round trips; bt0's bounce overlaps bt1's s-tree)
                    if STAGE < 1:
                        break

                    def bounce_bt(bt):
                        nc.sync.dma_start(
                            bounce_in[:, :].rearrange(
                                "(t b) x -> b t x", t=2, b=128)[:, bt],
                            s_sb[:, :].rearrange(
                                "b (t x) -> b t x", t=2, x=160)[:, bt])
                        if NO_CC:
                            nc.sync.dma_start(
                                bounce_out[bt * 128:(bt + 1) * 128, :],
                                bounce_in[bt * 128:(bt + 1) * 128, :])
                        else:
                            nc.gpsimd.collective_compute(
                                "AllReduce", ADD,
                                ins=[bounce_in[bt * 128:(bt + 1) * 128, :].opt()],
                                outs=[bounce_out[bt * 128:(bt + 1) * 128, :].opt()],
                                replica_groups=[list(range(CORES))],
                            )
                        nc.sync.dma_start(
                            s_sb[:, :].rearrange(
                                "b (t x) -> b t x", t=2, x=160)[:, bt],
                            bounce_out[:, :].rearrange(
                                "(t b) x -> b t x", t=2, b=128)[:, bt])

                    if r == 0:
                        bounce_bt(0)
                    bounce_bt(1)

                    if STAGE < 2:
                        break
                    # ---- alpha = n2 / ((n2+1)(sqrt(n2)+eps)) , n2 = sum s^2 ----
                    # squares+reduce on DVE (keeps ACT on one exp/ln table set)
                    nc.vector.tensor_tensor(ttr_junk[:, :], s_sb[:, :],
                                            s_sb[:, :], MULT)
                    nc.vector.tensor_reduce(
                        acc[:, :], ttr_junk[:, None, :],
                        mybir.AxisListType.X, ADD)
                    psn = psB.tile([128, 1], F32, tag="n2")
                    nc.tensor.matmul(psn[:, :], ones[:, :], acc[:, :],
                                     start=True, stop=True)
                    nc.vector.tensor_copy(n2sb[:, :], psn[:, :])
                    nc.vector.tensor_scalar_add(t1[:, :], n2sb[:, :], 1.0)
                    nc.vector.reciprocal(r1[:, :], t1[:, :])
                    nc.scalar.activation(lnv[:, :], n2sb[:, :], AF.Ln)
                    nc.scalar.activation(rt[:, :], lnv[:, :], AF.Exp, scale=0.5)
                    nc.vector.tensor_scalar_add(t2[:, :], rt[:, :], EPS)
                    nc.vector.reciprocal(r2[:, :], t2[:, :])
                    nc.vector.tensor_tensor(alpha_bc[:, :], n2sb[:, :], r1[:, :],
                                            MULT)
                    nc.vector.tensor_tensor(alpha_bc[:, :], alpha_bc[:, :],
                                            r2[:, :], MULT)

                    # ---- v = alpha * s (final output path) ----
                    nc.vector.tensor_scalar(sv[:, :], s_sb[:, :], alpha_bc[:, :],
                                            None, MULT)

                    if r < NROUT - 1 and STAGE >= 3:
                        # ==== g-phase in j-space ====
                        # s^T via PE transpose: rows (d,i) d-major, cols b.
                        nc.vector.tensor_copy(
                            s_bf[:, :].rearrange("p (t d i) -> p t d i",
                                                 t=2, d=10, i=32)[:, :, :, :16],
                            s_sb[:, :].rearrange("p (t i d) -> p t d i",
                                                 t=2, i=16, d=10))
                        for bt in range(2):
                            for rnd, (dlo, nd) in enumerate(
                                    ((0, 4), (4, 4), (8, 2))):
                                ptv = psT.tile([128, 128], BF16, tag="vt")
                                nc.tensor.transpose(
                                    ptv[:32 * nd, :],
                                    s_bf[:, bt * 320 + dlo * 32:
                                         bt * 320 + (dlo + nd) * 32],
                                    ident[:, :])
                                for dd in range(nd):
                                    nc.vector.tensor_copy(
                                        vtd[dlo + dd][:, bt * 128:
                                                      (bt + 1) * 128],
                                        ptv[32 * dd:32 * dd + 16, :])

                        for bt in range(2):
                            # wv[d] = s_d^T @ W_g[d]  (k=16), alpha in drain.
                            # Pool's digits (8,9) first so its g-chain starts
                            # early; DVE digits in pairs pipelined with drains.
                            wv_drain = [1, 1, 1, 0, 1, 1, 1, 0, 1, 1]
                            JP2 = J * PL // 3
                            wv4 = wv[:, :D * J * PL].rearrange(
                                "p (d j q) -> p d j q", d=D, j=J, q=PL)
                            xb = xbp[:, bt * (J * PL):(bt + 1) * (J * PL)]
                            xv = xb.rearrange("p (j q) -> p j q", j=J, q=PL)

                            def xview(dn):
                                return xv[:, None, :, :].to_broadcast(
                                    (128, dn, J, PL))

                            Ls = Lb[:, bt * 1440:(bt + 1) * 1440].rearrange(
                                "p (d q) -> p d q", d=D, q=PL)

                            def gfinal(eng, dlo, dn):
                                eng.tensor_tensor(
                                    wv4[:, dlo:dlo + dn], wv4[:, dlo:dlo + dn],
                                    xview(dn), MULT)
                                for sz in (4, 2, 1):
                                    eng.tensor_tensor(
                                        wv4[:, dlo:dlo + dn, 0:sz, :],
                                        wv4[:, dlo:dlo + dn, 0:sz, :],
                                        wv4[:, dlo:dlo + dn, sz:2 * sz, :],
                                        ADD)
                                eng.tensor_tensor(
                                    Ls[:, dlo:dlo + dn], Ls[:, dlo:dlo + dn],
                                    wv4[:, dlo:dlo + dn, 0, :], ADD)

                            d_order = [8, 9] + list(range(8))
                            for di, d in enumerate(d_order):
                                for h in range(3):
                                    psv = psW.tile([128, JP2], F32, tag="wv")
                                    nc.tensor.matmul(
                                        psv[:, :],
                                        vtd[d][:, bt * 128:(bt + 1) * 128],
                                        wg[:, d * (J * PL) + h * JP2:
                                           d * (J * PL) + (h + 1) * JP2],
                                        start=True, stop=True,
                                    )
                                    dstv = wv[:, d * (J * PL) + h * JP2:
                                              d * (J * PL) + (h + 1) * JP2]
                                    if wv_drain[d] == 0:
                                        nc.vector.tensor_scalar(
                                            dstv, psv[:, :], alpha_bc[:, :1],
                                            None, MULT)
                                    else:
                                        nc.scalar.mul(dstv, psv[:, :],
                                                      alpha_bc[:, :1])
                                if di == 1:
                                    gfinal(nc.gpsimd, DVE_D, POOL_D)
                                elif di >= 3 and di % 2 == 1:
                                    gfinal(nc.vector, d_order[di - 1], 2)

                            # softmax over d: exp writes (p,d)-transposed
                            nc.scalar.activation(
                                expL[:, :].rearrange("p (q d) -> p d q",
                                                     q=PL, d=D),
                                Lb[:, bt * 1440:(bt + 1) * 1440].rearrange(
                                    "p (d q) -> p d q", d=D, q=PL),
                                AF.Exp)
                            ev = expL[:, :].rearrange("p (q d) -> p q d",
                                                      q=PL, d=D)
                            # z-tree over d into zs scratch (keeps expL intact)
                            zv = zs[:, :].rearrange("p (q d) -> p q d",
                                                    q=PL, d=5)
                            nc.vector.tensor_tensor(
                                zv[:, :, :], ev[:, :, 0:5], ev[:, :, 5:10],
                                ADD)
                            nc.vector.tensor_tensor(
                                zv[:, :, 0:2], zv[:, :, 0:2], zv[:, :, 2:4],
                                ADD)
                            nc.vector.tensor_tensor(
                                zv[:, :, 0:1], zv[:, :, 0:1], zv[:, :, 1:2],
                                ADD)
                            nc.vector.tensor_tensor(
                                zv[:, :, 0:1], zv[:, :, 0:1], zv[:, :, 4:5],
                                ADD)
                            nc.vector.reciprocal(zrec[:, :, None], zv[:, :, 0:1])
                            nc.vector.tensor_tensor(
                                cpd[:, bt * 1440:(bt + 1) * 1440].rearrange(
                                    "p (q d) -> p q d", q=PL, d=D),
                                expL[:, :].rearrange("p (q d) -> p q d",
                                                     q=PL, d=D),
                                zrec[:, :, None].to_broadcast((128, PL, D)),
                                MULT)

                # ---- write v out:  v[b,d,i] = sv[b, i*10+d] ----
                vout_sb = pp.tile([128, 160], F32)
                vsrc = sv if STAGE >= 2 else s_sb
                for bt in range(2):
                    nc.vector.tensor_copy(
                        vout_sb[:, :].rearrange("p (d i) -> p d i", d=10, i=16),
                        vsrc[:, bt * 160:(bt + 1) * 160].rearrange(
                            "p (i d) -> p d i", i=16, d=10))
                    nc.sync.dma_start(v_d.ap()[bt * 128:(bt + 1) * 128, :, :],
                                      vout_sb[:, :])

    nc.compile()
    return nc


def prep_inputs(primary_caps: np.ndarray, W: np.ndarray):
    """Host-side shard + layout prep. Returns in_maps for the 8 cores."""
    x = np.asarray(primary_caps, dtype=np.float32)
    Wf = np.asarray(W, dtype=np.float32)
    bf = ml_dtypes.bfloat16
    ident = np.eye(128, dtype=np.float32).astype(bf)
    in_maps = []
    for k in range(CORES):
        pk = slice(k * PL, (k + 1) * PL)
        xk = x[:, pk, :]                       # [256, 144, 8]
        Wk = Wf[:, pk, :, :]                   # [10, 144, 16, 8]

        # x_l  [128, NQ*256]: row (gg*32 + ph*8 + j), col (q*256 + b)
        # group g = q*4+gg owns p_local = 3g..3g+2; ph==3 rows are padding
        xg = xk.reshape(B, G, 3, J)            # b, g, ph, j
        xl = np.zeros((G, 4, J, B), dtype=np.float32)
        xl[:, :3] = xg.transpose(1, 2, 3, 0)
        xl = xl.reshape(NQ, 4, 32, B).transpose(1, 2, 0, 3)  # gg, 32, q, b
        xl = xl.reshape(128, NQ * B)

        # w_bd [128, NQ*480]: row (gg*32 + ph*8 + j),
        #                     col (q*480 + php*160 + i*10 + d), delta(ph,php)
        Wt = Wk.reshape(D, G, 3, I, J).transpose(1, 2, 4, 3, 0)  # g,ph,j,i,d
        wbd = np.zeros((G, 4, J, 3, I, D), dtype=np.float32)
        for ph in range(3):
            wbd[:, ph, :, ph, :, :] = Wt[:, ph]
        wbd = wbd.reshape(NQ, 4, 32, GW).transpose(1, 2, 0, 3)
        wbd = wbd.reshape(128, NQ * GW)

        # x_pj [128, 9*256]: row = (p*8+j) % 128, col (kt*256 + b)
        xpj = xk.transpose(1, 2, 0).reshape(PL * J, B)
        xpj = xpj.reshape(NKT, 128, B).transpose(1, 0, 2).reshape(128, NKT * B)

        # w_fl [128, 9*160]: row = (p*8+j) % 128, col (kt*160 + i*10 + d)
        wfl = Wk.transpose(1, 3, 2, 0).reshape(PL * J, I * D)
        wfl = wfl.reshape(NKT, 128, I * D).transpose(1, 0, 2)
        wfl = wfl.reshape(128, NKT * I * D)

        # w_g [16, D*J*PL]: row i, col (d*1152 + j*144 + p) = W[d,p,i,j]
        wg = Wk.transpose(2, 0, 3, 1).reshape(I, D * J * PL)

        # x_bp [128, 2*J*PL]: row b%128, col (bt*1152 + j*144 + p)
        xbp = xk.transpose(0, 2, 1).reshape(B, J * PL)       # b, (j, p)
        xbp = xbp.reshape(2, 128, J * PL).transpose(1, 0, 2).reshape(
            128, 2 * J * PL)

        in_maps.append({
            "x_l": xl.astype(bf),
            "w_bd": wbd.astype(bf),
            "x_pj": xpj.astype(bf),
            "w_fl": wfl.astype(bf),
            "w_g": wg.astype(bf),
            "x_bp": xbp.astype(bf),
            "ident": ident,
        })
    return in_maps


_NC_CACHE = None


def get_program():
    global _NC_CACHE
    if _NC_CACHE is None:
        _NC_CACHE = build_program()
    return _NC_CACHE


def kernel(primary_caps: np.ndarray, W: np.ndarray) -> np.ndarray:
    nc = get_program()
    in_maps = prep_inputs(primary_caps, W)
    res = run_bass_kernel_spmd(nc, in_maps, core_ids=list(range(CORES)))
    return np.asarray(res.results[0]["v_out"], dtype=np.float32)


if __name__ == "__main__":
    rng = np.random.default_rng(0)
    x = rng.standard_normal((B, P, J), dtype=np.float32)
    W = rng.standard_normal((D, P, I, J), dtype=np.float32).astype(np.float32)
    out = kernel(x, W)
    print("out", out.shape, out.dtype, float(np.abs(out).mean()))



# revision 11
# speedup vs baseline: 1.2896x; 1.2896x over previous
"""CapsNet DigitCaps routing kernel for 8 TRN2 NeuronCores — v2.

Strategy: shard the 1152 primary capsules across the 8 cores (144 each),
keep the full batch (256, as two 128-row halves bt) on every core.

v2 never materializes u_hat. Per routing iteration:

  s-phase:   s[b,d,i] = sum_{p,j} W[d,p,i,j] * (c[b,d,p] * x[b,p,j])
    xc = c (*) x is a single broadcast multiply in j-space (8 wide, half
    the elements of c*u_hat); xc is PE-transposed per digit into (j,p)-
    partition tiles, drained to SBUF, and contracted with a host-packed
    W operand on the TensorEngine (9 accumulating K=128, N=16 matmuls
    per (bt,d)) straight into s[b, (d,i)] layout.
    At r=0, c == 0.1 uniformly, so the transposed x (times 0.1) is a
    static input and the s-phase is matmuls only.

  squash:    global-Frobenius-norm alpha; the per-core partial s is
    AllReduced (per-bt round trips so bt0's collective overlaps bt1's
    compute), then every core computes the same
    alpha = n2 / ((n2+1)(sqrt(n2)+eps)).

  g-phase (j-space): wv[b,d,p,j] = sum_i W[d,p,i,j] * s[b,d,i] on the
    TensorEngine (lhsT = s^T tiles from a post-AllReduce PE transpose),
    output kept bf16 in PSUM; g_raw = sum_j x (*) wv via a DVE/Pool
    multiply that reads PSUM directly plus a j-tree. alpha is never
    applied to wv or g_raw: the routing state is kept as
    E = prod_r exp(alpha_r * g_raw_r), updated with exp(scale=alpha)
    on the Activation engine and one bf16 multiply — softmax c is then
    E * (1/sum_d E).

Layouts (per core, SBUF partition dim first, p = local primary index):
  xbp [128, 2*1152] bf16   col = bt*1152 + j*144 + p      (g multiply)
  xt9 [128, 9*256]  bf16   0.1*x^T: row = (j*144+p)%128, col = k*256+b
  wsc [128, 1440]   bf16   row = (j*144+p)%128, col = (d*9+k)*16 + i
  wg  [16, 11520]   bf16   row = i, col = d*1152 + j*144 + p
  s_sb [128, 2*160] f32    col = bt*160 + d*16 + i  (matches v_out!)
  E,G,c [128, *1440] bf16  col = d*144 + p (d-major)
  xc  [128, 11520]  bf16   col = d*1152 + j*144 + p (per bt)
  xcT [128, 11520]  bf16   col = (d*9+k)*128 + (jp%128), value xc^T
  vtd [16, 2560]    bf16   col = d*256 + bt*128 + b  (s^T for wv lhsT)
"""

import os
import sys

for _p in ("/opt/trn_rl_repo",):
    if _p not in sys.path and os.path.isdir(_p):
        sys.path.insert(0, _p)

import numpy as np
import ml_dtypes

import concourse.bass as bass
import concourse.bacc as bacc
import concourse.mybir as mybir
import concourse.tile as tile
from concourse.bass_utils import run_bass_kernel_spmd

F32 = mybir.dt.float32
BF16 = mybir.dt.bfloat16
MULT = mybir.AluOpType.mult
ADD = mybir.AluOpType.add
AF = mybir.ActivationFunctionType

B, D, P, I, J = 256, 10, 1152, 16, 8
CORES = 8
PL = P // CORES          # 144 local primary capsules
JP = J * PL              # 1152 (j,p) rows per digit
NK = JP // 128           # 9 partition chunks per digit
EPS = 1e-7
NROUT = 3

# routing rounds actually executed (debug bisection: 1, 2, or 3)
STAGE = int(os.environ.get("BASSCAPS_STAGE", "3"))
# Replace the AllReduce with a local DRAM copy (for TimelineSim profiling).
NO_CC = os.environ.get("BASSCAPS_NO_CC", "0") == "1"
# Pool engine shares
POOL_XC_D = int(os.environ.get("BASSCAPS_POOL_XC_D", "2"))   # digits of xc mult
POOL_G_D = int(os.environ.get("BASSCAPS_POOL_G_D", "2"))     # digits of g phase
DVE_DIRECT_D = int(os.environ.get("BASSCAPS_DVE_DIRECT_D", "1"))
# of the 12 xcT drain batches per bt: how many go to ACT / Pool (rest DVE)
DRAIN_ACT = int(os.environ.get("BASSCAPS_DRAIN_ACT", "6"))


def build_program():
    nc = bacc.Bacc("TRN2", target_bir_lowering=False, debug=False,
                   num_devices=CORES)

    xbp_d = nc.dram_tensor("x_bp", [128, 2 * JP], BF16, kind="ExternalInput")
    xt9_d = nc.dram_tensor("x_t9", [128, NK * 256], BF16, kind="ExternalInput")
    wsc_d = nc.dram_tensor("w_sc", [128, D * NK * 16], BF16,
                           kind="ExternalInput")
    wg_d = nc.dram_tensor("w_g", [128, D * JP], BF16, kind="ExternalInput")
    ident_d = nc.dram_tensor("ident", [128, 128], BF16, kind="ExternalInput")
    v_d = nc.dram_tensor("v_out", [B, D, I], F32, kind="ExternalOutput")

    NROUT_RUN = max(1, min(NROUT, STAGE))

    with tile.TileContext(nc) as tc:
        with (
            tc.tile_pool(name="persist", bufs=1) as pp,
            tc.tile_pool(name="psT", bufs=3, space=bass.MemorySpace.PSUM) as psT,
            tc.tile_pool(name="psS", bufs=1, space=bass.MemorySpace.PSUM) as psS,
            tc.tile_pool(name="psW", bufs=2, space=bass.MemorySpace.PSUM) as psW,
            tc.tile_pool(name="psV", bufs=1, space=bass.MemorySpace.PSUM) as psV,
            tc.tile_pool(name="dram", bufs=1, space=bass.MemorySpace.DRAM) as dp,
        ):
            xbp = pp.tile([128, 2 * JP], BF16)
            xt9 = pp.tile([128, NK * 256], BF16)
            wsc = pp.tile([128, D * NK * 16], BF16)
            wg = pp.tile([128, D * JP], BF16)
            ident = pp.tile([128, 128], BF16)

            s_sb = pp.tile([128, 2 * 160], F32)
            sv = pp.tile([128, 2 * 160], F32)
            s_bf = pp.tile([128, 2 * 320], BF16)   # pad-32 staging for vt
            vtd = pp.tile([128, 4 * 256], BF16)
            E = pp.tile([128, 2 * 1440], BF16)
            Et = pp.tile([128, 2 * 1440], BF16)
            G = pp.tile([128, 2 * 1440], BF16)
            cb = pp.tile([128, 2 * 1440], BF16)
            zs = pp.tile([128, 2 * 720], BF16)
            zrec = pp.tile([128, 2 * 144], F32)
            zrecb = pp.tile([128, 2 * 144], BF16)
            xc = pp.tile([128, D * JP], BF16)
            xcT = pp.tile([128, D * JP], BF16)
            Y = pp.tile([128, 2 * JP], BF16)       # g-phase scratch (2 bufs)
            Yw = pp.tile([128, 2 * JP], BF16)      # wv drain staging

            sq = pp.tile([128, 2 * 160], F32)
            accb = pp.tile([128, 2], F32)
            acc = pp.tile([128, 1], F32)
            ones = pp.tile([128, 128], F32)
            n2sb = pp.tile([128, 1], F32)
            t1 = pp.tile([128, 1], F32)
            r1 = pp.tile([128, 1], F32)
            lnv = pp.tile([128, 1], F32)
            rt = pp.tile([128, 1], F32)
            t2 = pp.tile([128, 1], F32)
            r2 = pp.tile([128, 1], F32)
            alpha_bc = pp.tile([128, 1], F32)

            bounce_in = dp.tile([B, 160], F32)
            bounce_out = dp.tile([B, 160], F32)

            nc.vector.memset(ones[:, :], 1.0)
            # warm the ACT exp/ln table set
            nc.scalar.activation(t1[:, :], ones[:, :1], AF.Exp)
            nc.scalar.activation(t2[:, :], ones[:, :1], AF.Ln)

            nc.sync.dma_start(xt9[:, :], xt9_d.ap())
            nc.sync.dma_start(wsc[:, :], wsc_d.ap())
            nc.sync.dma_start(ident[:, :], ident_d.ap())
            nc.sync.dma_start(xbp[:, :], xbp_d.ap())
            nc.sync.dma_start(wg[:, :], wg_d.ap())

            # ---------------- helpers ----------------

            def s_contract(r, bt):
                """s_partial[b, (d,i)] for batch-half bt -> s_sb."""
                if r > 0:
                    # xc = c (*) x  (cols d*1152 + j*144 + p), split by d
                    dpool = POOL_XC_D
                    xc4 = xc[:, :].rearrange("m (d j p) -> m d j p",
                                             d=D, j=J, p=PL)
                    cv = cb[:, bt * 1440:(bt + 1) * 1440].rearrange(
                        "m (d p) -> m d p", d=D, p=PL)
                    xv = xbp[:, bt * JP:(bt + 1) * JP].rearrange(
                        "m (j p) -> m j p", j=J, p=PL)
                    for eng, dlo, dn in ((nc.vector, 0, D - dpool),
                                         (nc.gpsimd, D - dpool, dpool)):
                        if dn == 0:
                            continue
                        eng.tensor_tensor(
                            xc4[:, dlo:dlo + dn],
                            cv[:, dlo:dlo + dn, None, :].to_broadcast(
                                (128, dn, J, PL)),
                            xv[:, None, :, :].to_broadcast((128, dn, J, PL)),
                            MULT)
                    # transpose xc -> xcT (90 tiles of 128 cols) with
                    # bank-packed drains of 8 tiles (1024 cols)
                    nbatch = (D * NK + 7) // 8     # 12 (last has 2 tiles)
                    drain_eng = ([nc.scalar] * DRAIN_ACT
                                 + [nc.vector] * nbatch)[:nbatch]
                    for batch in range(nbatch):
                        tlo = batch * 8
                        tn = min(8, D * NK - tlo)
                        pt = psT.tile([128, 1024], BF16, tag="tr")
                        for t in range(tn):
                            nc.tensor.transpose(
                                pt[:, t * 128:(t + 1) * 128],
                                xc[:, (tlo + t) * 128:(tlo + t + 1) * 128],
                                ident[:, :])
                        eng = drain_eng[batch]
                        dst = xcT[:, tlo * 128:(tlo + tn) * 128]
                        if eng is nc.scalar:
                            eng.copy(dst, pt[:, :tn * 128])
                        else:
                            eng.tensor_copy(dst, pt[:, :tn * 128])

                ps = psS.tile([128, 160], F32, tag="s")
                for d in range(D):
                    for k in range(NK):
                        if r == 0:
                            lhsT = xt9[:, k * 256 + bt * 128:
                                       k * 256 + bt * 128 + 128]
                        else:
                            lhsT = xcT[:, (d * NK + k) * 128:
                                       (d * NK + k + 1) * 128]
                        nc.tensor.matmul(
                            ps[:, d * 16:(d + 1) * 16],
                            lhsT,
                            wsc[:, (d * NK + k) * 16:(d * NK + k + 1) * 16],
                            start=(k == 0), stop=(k == NK - 1))
                nc.vector.tensor_copy(s_sb[:, bt * 160:(bt + 1) * 160],
                                      ps[:, :])

            def bounce_bt(bt):
                nc.sync.dma_start(
                    bounce_in[bt * 128:(bt + 1) * 128, :],
                    s_sb[:, bt * 160:(bt + 1) * 160])
                if NO_CC:
                    nc.sync.dma_start(
                        bounce_out[bt * 128:(bt + 1) * 128, :],
                        bounce_in[bt * 128:(bt + 1) * 128, :])
                else:
                    nc.gpsimd.collective_compute(
                        "AllReduce", ADD,
                        ins=[bounce_in[bt * 128:(bt + 1) * 128, :].opt()],
                        outs=[bounce_out[bt * 128:(bt + 1) * 128, :].opt()],
                        replica_groups=[list(range(CORES))],
                    )
                nc.sync.dma_start(
                    s_sb[:, bt * 160:(bt + 1) * 160],
                    bounce_out[bt * 128:(bt + 1) * 128, :])

            def n2_partial(bt):
                ss = s_sb[:, bt * 160:(bt + 1) * 160]
                nc.vector.tensor_tensor(sq[:, bt * 160:(bt + 1) * 160],
                                        ss, ss, MULT)
                nc.vector.tensor_reduce(
                    accb[:, bt:bt + 1],
                    sq[:, None, bt * 160:(bt + 1) * 160],
                    mybir.AxisListType.X, ADD)

            def alpha_final():
                # alpha = n2 / ((n2+1)(sqrt(n2)+eps)); sqrt via ln/exp so
                # ACT stays on the exp/ln table set
                nc.vector.tensor_tensor(acc[:, :], accb[:, 0:1], accb[:, 1:2],
                                        ADD)
                psn = psV.tile([128, 1], F32, tag="n2")
                nc.tensor.matmul(psn[:, :], ones[:, :], acc[:, :],
                                 start=True, stop=True)
                nc.vector.tensor_copy(n2sb[:, :], psn[:, :])
                nc.vector.tensor_scalar_add(t1[:, :], n2sb[:, :], 1.0)
                nc.vector.reciprocal(r1[:, :], t1[:, :])
                nc.scalar.activation(lnv[:, :], n2sb[:, :], AF.Ln)
                nc.scalar.activation(rt[:, :], lnv[:, :], AF.Exp, scale=0.5)
                nc.vector.tensor_scalar_add(t2[:, :], rt[:, :], EPS)
                nc.vector.reciprocal(r2[:, :], t2[:, :])
                nc.vector.tensor_tensor(alpha_bc[:, :], n2sb[:, :], r1[:, :],
                                        MULT)
                nc.vector.tensor_tensor(alpha_bc[:, :], alpha_bc[:, :],
                                        r2[:, :], MULT)

            def vtd_path(bt):
                # s^T tiles for the wv matmul lhsT (post-AllReduce s).
                # vtd row layout: digit d lives at partitions 32*(d%4)..+16
                # of round d//4; lhsT base partitions are 0/32/64/96.
                nc.vector.tensor_copy(
                    s_bf[:, bt * 320:(bt + 1) * 320].rearrange(
                        "m (d i) -> m d i", d=D, i=32)[:, :, :16],
                    s_sb[:, bt * 160:(bt + 1) * 160].rearrange(
                        "m (d i) -> m d i", d=D, i=16))
                for rnd, (dlo, nd) in enumerate(((0, 3), (3, 3), (6, 3),
                                                 (9, 1))):
                    ptv = psV.tile([128, 128], BF16, tag="vt")
                    nc.tensor.transpose(
                        ptv[:32 * nd, :],
                        s_bf[:, bt * 320 + dlo * 32:
                             bt * 320 + (dlo + nd) * 32],
                        ident[:, :])
                    nc.vector.tensor_copy(
                        vtd[:32 * nd, rnd * 256 + bt * 128:
                            rnd * 256 + bt * 128 + 128],
                        ptv[:32 * nd, :])

            def vt_ap(d, bt):
                return vtd[32 * (d % 3):32 * (d % 3) + 16,
                           (d // 3) * 256 + bt * 128:
                           (d // 3) * 256 + bt * 128 + 128]

            # per-digit g-phase mode: ACT drain + DVE mult, ACT drain +
            # Pool mult, or DVE mult reading PSUM f32 directly (1x)
            G_MODES = (["ad"] * (D - POOL_G_D - DVE_DIRECT_D)
                       + ["dv"] * DVE_DIRECT_D + ["ap"] * POOL_G_D)

            def g_phase(bt):
                # G[b, (d,p)] = sum_j x * (W @ s^T)   (raw, no alpha)
                for d in range(D):
                    mode = G_MODES[d]
                    eng = nc.gpsimd if mode == "ap" else nc.vector
                    yb = Y[:, (d % 2) * JP:(d % 2) * JP + JP]
                    ywb = Yw[:, (d % 2) * JP:(d % 2) * JP + JP]
                    for h in range(3):
                        pw = psW.tile([128, 384], F32, tag="wv")
                        q = 32 * (d % 3)
                        nc.tensor.matmul(
                            pw[:, :], vt_ap(d, bt),
                            wg[q:q + 16,
                               d * JP + h * 384:d * JP + (h + 1) * 384],
                            start=True, stop=True)
                        xs = xbp[:, bt * JP + h * 384:bt * JP + (h + 1) * 384]
                        if mode == "dv":
                            nc.vector.tensor_tensor(
                                yb[:, h * 384:(h + 1) * 384], pw[:, :], xs,
                                MULT)
                        else:
                            nc.scalar.copy(ywb[:, h * 384:(h + 1) * 384],
                                           pw[:, :])
                            eng.tensor_tensor(
                                yb[:, h * 384:(h + 1) * 384],
                                ywb[:, h * 384:(h + 1) * 384], xs, MULT)
                    eng.tensor_tensor(yb[:, 0:576], yb[:, 0:576],
                                      yb[:, 576:1152], ADD)
                    eng.tensor_tensor(yb[:, 0:288], yb[:, 0:288],
                                      yb[:, 288:576], ADD)
                    eng.tensor_tensor(
                        G[:, bt * 1440 + d * 144:bt * 1440 + (d + 1) * 144],
                        yb[:, 0:144], yb[:, 144:288], ADD)

            def e_update(r, bt):
                gb = G[:, bt * 1440:(bt + 1) * 1440]
                eb = E[:, bt * 1440:(bt + 1) * 1440]
                if r == 0:
                    nc.scalar.activation(eb, gb, AF.Exp, scale=alpha_bc[:, :1])
                else:
                    etb = Et[:, bt * 1440:(bt + 1) * 1440]
                    nc.scalar.activation(etb, gb, AF.Exp,
                                         scale=alpha_bc[:, :1])
                    nc.vector.tensor_tensor(eb, eb, etb, MULT)

            def softmax_c(bt):
                eb = E[:, bt * 1440:(bt + 1) * 1440]
                zb = zs[:, bt * 720:(bt + 1) * 720]
                nc.vector.tensor_tensor(zb[:, 0:720], eb[:, 0:720],
                                        eb[:, 720:1440], ADD)
                nc.vector.tensor_tensor(zb[:, 0:288], zb[:, 0:288],
                                        zb[:, 288:576], ADD)
                nc.vector.tensor_tensor(zb[:, 0:144], zb[:, 0:144],
                                        zb[:, 144:288], ADD)
                nc.vector.tensor_tensor(zb[:, 0:144], zb[:, 0:144],
                                        zb[:, 576:720], ADD)
                nc.vector.reciprocal(zrec[:, bt * 144:(bt + 1) * 144],
                                     zb[:, 0:144])
                nc.vector.tensor_copy(zrecb[:, bt * 144:(bt + 1) * 144],
                                      zrec[:, bt * 144:(bt + 1) * 144])
                nc.vector.tensor_tensor(
                    cb[:, bt * 1440:(bt + 1) * 1440].rearrange(
                        "m (d p) -> m d p", d=D, p=PL),
                    eb.rearrange("m (d p) -> m d p", d=D, p=PL),
                    zrecb[:, None, bt * 144:(bt + 1) * 144].to_broadcast(
                        (128, D, PL)),
                    MULT)

            # ---------------- schedule ----------------
            for r in range(NROUT_RUN):
                for bt in range(2):
                    if r > 0:
                        softmax_c(bt)
                    s_contract(r, bt)
                    bounce_bt(bt)
                    n2_partial(bt)
                alpha_final()
                if r < NROUT - 1 and r < NROUT_RUN - 1:
                    for bt in range(2):
                        vtd_path(bt)
                        g_phase(bt)
                        e_update(r, bt)

            # ---- v = alpha * s ; layout already matches v_out ----
            for bt in range(2):
                nc.vector.tensor_scalar(sv[:, bt * 160:(bt + 1) * 160],
                                        s_sb[:, bt * 160:(bt + 1) * 160],
                                        alpha_bc[:, :1], None, MULT)
                nc.sync.dma_start(v_d.ap()[bt * 128:(bt + 1) * 128, :, :],
                                  sv[:, bt * 160:(bt + 1) * 160])

    nc.compile()
    return nc


def prep_inputs(primary_caps: np.ndarray, W: np.ndarray):
    """Host-side shard + layout prep. Returns in_maps for the 8 cores."""
    x = np.asarray(primary_caps, dtype=np.float32)
    Wf = np.asarray(W, dtype=np.float32)
    bf = ml_dtypes.bfloat16
    ident = np.eye(128, dtype=np.float32).astype(bf)
    in_maps = []
    for kcore in range(CORES):
        pk = slice(kcore * PL, (kcore + 1) * PL)
        xk = x[:, pk, :]                       # [256, 144, 8]
        Wk = Wf[:, pk, :, :]                   # [10, 144, 16, 8]

        # x_bp [128, 2*1152]: row b%128, col (bt*1152 + j*144 + p)
        xbp = xk.transpose(0, 2, 1).reshape(B, JP)
        xbp = xbp.reshape(2, 128, JP).transpose(1, 0, 2).reshape(128, 2 * JP)

        # x_t9 [128, 9*256]: 0.1*x^T: row (j*144+p)%128, col k*256 + b
        xt9 = 0.1 * xk.transpose(2, 1, 0).reshape(JP, B)
        xt9 = xt9.reshape(NK, 128, B).transpose(1, 0, 2).reshape(128, NK * B)

        # w_sc [128, 10*9*16]: row (j*144+p)%128, col (d*9+k)*16 + i
        wsc = Wk.transpose(0, 3, 1, 2).reshape(D, JP, I)
        wsc = wsc.reshape(D, NK, 128, I).transpose(2, 0, 1, 3)
        wsc = wsc.reshape(128, D * NK * I)

        # w_g [128, D*J*PL]: rows 32q+i (4 replicas), col d*1152 + j*144 + p
        wg1 = Wk.transpose(2, 0, 3, 1).reshape(I, D * JP)
        wg = np.zeros((128, D * JP), dtype=np.float32)
        for q in range(3):
            wg[32 * q:32 * q + I] = wg1

        in_maps.append({
            "x_bp": xbp.astype(bf),
            "x_t9": xt9.astype(bf),
            "w_sc": wsc.astype(bf),
            "w_g": wg.astype(bf),
            "ident": ident,
        })
    return in_maps


_NC_CACHE = None


def get_program():
    global _NC_CACHE
    if _NC_CACHE is None:
        _NC_CACHE = build_program()
    return _NC_CACHE


def kernel(primary_caps: np.ndarray, W: np.ndarray) -> np.ndarray:
    nc = get_program()
    in_maps = prep_inputs(primary_caps, W)
    res = run_bass_kernel_spmd(nc, in_maps, core_ids=list(range(CORES)))
    return np.asarray(res.results[0]["v_out"], dtype=np.float32)


if __name__ == "__main__":
    rng = np.random.default_rng(0)
    x = rng.standard_normal((B, P, J), dtype=np.float32)
    W = rng.standard_normal((D, P, I, J), dtype=np.float32).astype(np.float32)
    out = kernel(x, W)
    print("out", out.shape, out.dtype, float(np.abs(out).mean()))


# revision 15
# speedup vs baseline: 1.4442x; 1.1198x over previous
"""CapsNet DigitCaps routing kernel for 8 TRN2 NeuronCores — v2.

Strategy: shard the 1152 primary capsules across the 8 cores (144 each),
keep the full batch (256, as two 128-row halves bt) on every core.

v2 never materializes u_hat. Per routing iteration:

  s-phase:   s[b,d,i] = sum_{p,j} W[d,p,i,j] * (c[b,d,p] * x[b,p,j])
    xc = c (*) x is a single broadcast multiply in j-space (8 wide, half
    the elements of c*u_hat); xc is PE-transposed per digit into (j,p)-
    partition tiles, drained to SBUF, and contracted with a host-packed
    W operand on the TensorEngine (9 accumulating K=128, N=16 matmuls
    per (bt,d)) straight into s[b, (d,i)] layout.
    At r=0, c == 0.1 uniformly, so the transposed x (times 0.1) is a
    static input and the s-phase is matmuls only.

  squash:    global-Frobenius-norm alpha; the per-core partial s is
    AllReduced (per-bt round trips so bt0's collective overlaps bt1's
    compute), then every core computes the same
    alpha = n2 / ((n2+1)(sqrt(n2)+eps)).

  g-phase (j-space): wv[b,d,p,j] = sum_i W[d,p,i,j] * s[b,d,i] on the
    TensorEngine (lhsT = s^T tiles from a post-AllReduce PE transpose),
    output kept bf16 in PSUM; g_raw = sum_j x (*) wv via a DVE/Pool
    multiply that reads PSUM directly plus a j-tree. alpha is never
    applied to wv or g_raw: the routing state is kept as
    E = prod_r exp(alpha_r * g_raw_r), updated with exp(scale=alpha)
    on the Activation engine and one bf16 multiply — softmax c is then
    E * (1/sum_d E).

Layouts (per core, SBUF partition dim first, p = local primary index):
  xbp [128, 2*1152] bf16   col = bt*1152 + j*144 + p      (g multiply)
  xt9 [128, 9*256]  bf16   0.1*x^T: row = (j*144+p)%128, col = k*256+b
  wsc [128, 1440]   bf16   row = (j*144+p)%128, col = (d*9+k)*16 + i
  wg  [16, 11520]   bf16   row = i, col = d*1152 + j*144 + p
  s_sb [128, 2*160] f32    col = bt*160 + d*16 + i  (matches v_out!)
  E,G,c [128, *1440] bf16  col = d*144 + p (d-major)
  xc  [128, 11520]  bf16   col = d*1152 + j*144 + p (per bt)
  xcT [128, 11520]  bf16   col = (d*9+k)*128 + (jp%128), value xc^T
  vtd [16, 2560]    bf16   col = d*256 + bt*128 + b  (s^T for wv lhsT)
"""

import os
import sys

for _p in ("/opt/trn_rl_repo",):
    if _p not in sys.path and os.path.isdir(_p):
        sys.path.insert(0, _p)

import numpy as np
import ml_dtypes

import concourse.bass as bass
import concourse.bacc as bacc
import concourse.mybir as mybir
import concourse.tile as tile
from concourse.bass_utils import run_bass_kernel_spmd

F32 = mybir.dt.float32
BF16 = mybir.dt.bfloat16
MULT = mybir.AluOpType.mult
ADD = mybir.AluOpType.add
AF = mybir.ActivationFunctionType

B, D, P, I, J = 256, 10, 1152, 16, 8
CORES = 8
PL = P // CORES          # 144 local primary capsules
JP = J * PL              # 1152 (j,p) rows per digit
NK = JP // 128           # 9 partition chunks per digit
EPS = 1e-7
NROUT = 3

# routing rounds actually executed (debug bisection: 1, 2, or 3)
STAGE = int(os.environ.get("BASSCAPS_STAGE", "3"))
# Replace the AllReduce with a local DRAM copy (for TimelineSim profiling).
NO_CC = os.environ.get("BASSCAPS_NO_CC", "0") == "1"
# Pool engine shares
POOL_XC_D = int(os.environ.get("BASSCAPS_POOL_XC_D", "2"))   # digits of xc mult
POOL_G_D = int(os.environ.get("BASSCAPS_POOL_G_D", "2"))     # digits of g phase
DVE_DIRECT_D = int(os.environ.get("BASSCAPS_DVE_DIRECT_D", "1"))
# of the 12 xcT drain batches per bt: how many go to ACT / Pool (rest DVE)
DRAIN_ACT = int(os.environ.get("BASSCAPS_DRAIN_ACT", "6"))


def build_program():
    nc = bacc.Bacc("TRN2", target_bir_lowering=False, debug=False,
                   num_devices=CORES)

    xbp_d = nc.dram_tensor("x_bp", [128, 2 * JP], BF16, kind="ExternalInput")
    xt9_d = nc.dram_tensor("x_t9", [128, NK * 256], BF16, kind="ExternalInput")
    wsc_d = nc.dram_tensor("w_sc", [128, D * NK * 16], BF16,
                           kind="ExternalInput")
    wg_d = nc.dram_tensor("w_g", [128, D * JP], BF16, kind="ExternalInput")
    ident_d = nc.dram_tensor("ident", [128, 128], BF16, kind="ExternalInput")
    v_d = nc.dram_tensor("v_out", [B, D, I], F32, kind="ExternalOutput")

    NROUT_RUN = max(1, min(NROUT, STAGE))

    with tile.TileContext(nc) as tc:
        with (
            tc.tile_pool(name="persist", bufs=1) as pp,
            tc.tile_pool(name="psT", bufs=4, space=bass.MemorySpace.PSUM) as psT,
            tc.tile_pool(name="psS", bufs=1, space=bass.MemorySpace.PSUM) as psS,
            tc.tile_pool(name="psW", bufs=2, space=bass.MemorySpace.PSUM) as psW,
            tc.tile_pool(name="psV", bufs=1, space=bass.MemorySpace.PSUM) as psV,
            tc.tile_pool(name="dram", bufs=1, space=bass.MemorySpace.DRAM) as dp,
        ):
            xbp = pp.tile([128, 2 * JP], BF16)
            xt9 = pp.tile([128, NK * 256], BF16)
            wsc = pp.tile([128, D * NK * 16], BF16)
            wg = pp.tile([128, D * JP], BF16)
            ident = pp.tile([128, 128], BF16)

            s_sb = pp.tile([128, 2 * 160], F32)
            sv = pp.tile([128, 2 * 160], F32)
            s_bf = pp.tile([128, 2 * 320], BF16)   # pad-32 staging for vt
            vtd = pp.tile([128, 4 * 256], BF16)
            E = pp.tile([128, 2 * 1440], BF16)
            Et = pp.tile([128, 2 * 1440], BF16)
            G = pp.tile([128, 2 * 1440], BF16)
            cb = pp.tile([128, 2 * 1440], BF16)
            zs = pp.tile([128, 2 * 720], BF16)
            zrec = pp.tile([128, 2 * 144], F32)
            zrecb = pp.tile([128, 2 * 144], BF16)
            xc = pp.tile([128, D * JP], BF16)
            xcT = pp.tile([128, D * JP], BF16)
            Y = pp.tile([128, 2 * JP], BF16)       # g-phase scratch (2 bufs)
            Yw = pp.tile([128, 2 * JP], BF16)      # wv drain staging

            sq = pp.tile([128, 2 * 160], F32)
            accb = pp.tile([128, 2], F32)
            acc = pp.tile([128, 1], F32)
            ones = pp.tile([128, 128], F32)
            n2sb = pp.tile([128, 1], F32)
            t1 = pp.tile([128, 1], F32)
            r1 = pp.tile([128, 1], F32)
            lnv = pp.tile([128, 1], F32)
            rt = pp.tile([128, 1], F32)
            t2 = pp.tile([128, 1], F32)
            r2 = pp.tile([128, 1], F32)
            alpha_bc = pp.tile([128, 1], F32)

            bounce_in = dp.tile([B, 160], F32)
            bounce_out = dp.tile([B, 160], F32)

            nc.vector.memset(ones[:, :], 1.0)
            # warm the ACT exp/ln table set
            nc.scalar.activation(t1[:, :], ones[:, :1], AF.Exp)
            nc.scalar.activation(t2[:, :], ones[:, :1], AF.Ln)

            nc.sync.dma_start(xt9[:, :], xt9_d.ap())
            nc.sync.dma_start(wsc[:, :], wsc_d.ap())
            nc.sync.dma_start(ident[:, :], ident_d.ap())
            nc.sync.dma_start(xbp[:, :], xbp_d.ap())
            nc.sync.dma_start(wg[:, :], wg_d.ap())

            # ---------------- helpers ----------------

            def xc_chunk(bt, dlo, dn, eng):
                """xc[:, dlo:dlo+dn digits] = c (*) x for batch-half bt."""
                xc4 = xc[:, :].rearrange("m (d j p) -> m d j p",
                                         d=D, j=J, p=PL)
                cv = cb[:, bt * 1440:(bt + 1) * 1440].rearrange(
                    "m (d p) -> m d p", d=D, p=PL)
                xv = xbp[:, bt * JP:(bt + 1) * JP].rearrange(
                    "m (j p) -> m j p", j=J, p=PL)
                eng.tensor_tensor(
                    xc4[:, dlo:dlo + dn],
                    cv[:, dlo:dlo + dn, None, :].to_broadcast(
                        (128, dn, J, PL)),
                    xv[:, None, :, :].to_broadcast((128, dn, J, PL)),
                    MULT)

            def s_matmuls(r, bt, ps, dlo, dn):
                for d in range(dlo, dlo + dn):
                    for k in range(NK):
                        if r == 0:
                            lhsT = xt9[:, k * 256 + bt * 128:
                                       k * 256 + bt * 128 + 128]
                        else:
                            lhsT = xcT[:, (d * NK + k) * 128:
                                       (d * NK + k + 1) * 128]
                        nc.tensor.matmul(
                            ps[:, d * 16:(d + 1) * 16],
                            lhsT,
                            wsc[:, (d * NK + k) * 16:(d * NK + k + 1) * 16],
                            start=(k == 0), stop=(k == NK - 1))

            def s_contract(r, bt):
                """s_partial[b, (d,i)] for batch-half bt -> s_sb.

                For r>0 the xc multiply, the PE transposes, their drains,
                and the contraction matmuls are emitted interleaved so the
                in-order engine queues pipeline across digit groups.
                """
                ps = psS.tile([128, 176], F32, tag="s")
                if r == 0:
                    s_matmuls(0, bt, ps, 0, D)
                else:
                    # xc chunks by digit group; group 3 goes to Pool
                    XCG = ((0, 3, nc.vector), (3, 3, nc.vector),
                           (6, 2, nc.gpsimd), (8, 2, nc.vector))
                    emitted_xc = 0
                    drained = 0          # tiles drained to xcT so far
                    mm_done = 0          # digits contracted so far
                    nbatch = (D * NK + 7) // 8
                    dr_i = 0
                    for batch in range(nbatch):
                        tlo = batch * 8
                        tn = min(8, D * NK - tlo)
                        # ensure xc digits covering tiles [tlo, tlo+tn) exist
                        need_d = min(D, (tlo + tn + NK - 1) // NK)
                        while emitted_xc < len(XCG):
                            glo, gn, geng = XCG[emitted_xc]
                            if glo >= need_d:
                                break
                            xc_chunk(bt, glo, gn, geng)
                            emitted_xc += 1
                        pt = psT.tile([128, 1024], BF16, tag="tr")
                        for t in range(tn):
                            nc.tensor.transpose(
                                pt[:, t * 128:(t + 1) * 128],
                                xc[:, (tlo + t) * 128:(tlo + t + 1) * 128],
                                ident[:, :])
                        eng = nc.scalar if dr_i % 2 == 0 else nc.vector
                        dr_i += 1
                        dst = xcT[:, tlo * 128:(tlo + tn) * 128]
                        if eng is nc.scalar:
                            eng.copy(dst, pt[:, :tn * 128])
                        else:
                            eng.tensor_copy(dst, pt[:, :tn * 128])
                        drained = tlo + tn
                        # contract digits whose 9 tiles are all drained
                        nd = drained // NK - mm_done
                        if nd > 0:
                            s_matmuls(r, bt, ps, mm_done, nd)
                            mm_done += nd
                    if mm_done < D:
                        s_matmuls(r, bt, ps, mm_done, D - mm_done)
                nc.vector.tensor_copy(s_sb[:, bt * 160:(bt + 1) * 160],
                                      ps[:, :160])
                return ps

            def bounce_bt(bt):
                nc.sync.dma_start(
                    bounce_in[bt * 128:(bt + 1) * 128, :],
                    s_sb[:, bt * 160:(bt + 1) * 160])
                if NO_CC:
                    nc.sync.dma_start(
                        bounce_out[bt * 128:(bt + 1) * 128, :],
                        bounce_in[bt * 128:(bt + 1) * 128, :])
                else:
                    nc.gpsimd.collective_compute(
                        "AllReduce", ADD,
                        ins=[bounce_in[bt * 128:(bt + 1) * 128, :].opt()],
                        outs=[bounce_out[bt * 128:(bt + 1) * 128, :].opt()],
                        replica_groups=[list(range(CORES))],
                    )
                nc.sync.dma_start(
                    s_sb[:, bt * 160:(bt + 1) * 160],
                    bounce_out[bt * 128:(bt + 1) * 128, :])

            def n2_partial(bt):
                ss = s_sb[:, bt * 160:(bt + 1) * 160]
                nc.vector.tensor_tensor(sq[:, bt * 160:(bt + 1) * 160],
                                        ss, ss, MULT)
                nc.vector.tensor_reduce(
                    accb[:, bt:bt + 1],
                    sq[:, None, bt * 160:(bt + 1) * 160],
                    mybir.AxisListType.X, ADD)

            def alpha_final(psn):
                # alpha = n2 / ((n2+1)(sqrt(n2)+eps)); sqrt via ln/exp so
                # ACT stays on the exp/ln table set. psn: spare cols
                # [160:161] of the last s-contract PSUM tile.
                nc.vector.tensor_tensor(acc[:, :], accb[:, 0:1], accb[:, 1:2],
                                        ADD)
                nc.tensor.matmul(psn[:, 160:161], ones[:, :], acc[:, :],
                                 start=True, stop=True)
                nc.vector.tensor_copy(n2sb[:, :], psn[:, 160:161])
                nc.vector.tensor_scalar_add(t1[:, :], n2sb[:, :], 1.0)
                nc.vector.reciprocal(r1[:, :], t1[:, :])
                nc.scalar.activation(lnv[:, :], n2sb[:, :], AF.Ln)
                nc.scalar.activation(rt[:, :], lnv[:, :], AF.Exp, scale=0.5)
                nc.vector.tensor_scalar_add(t2[:, :], rt[:, :], EPS)
                nc.vector.reciprocal(r2[:, :], t2[:, :])
                nc.vector.tensor_tensor(alpha_bc[:, :], n2sb[:, :], r1[:, :],
                                        MULT)
                nc.vector.tensor_tensor(alpha_bc[:, :], alpha_bc[:, :],
                                        r2[:, :], MULT)

            def vtd_path(bt):
                # s^T tiles for the wv matmul lhsT (post-AllReduce s).
                # vtd row layout: digit d lives at partitions 32*(d%4)..+16
                # of round d//4; lhsT base partitions are 0/32/64/96.
                nc.vector.tensor_copy(
                    s_bf[:, bt * 320:(bt + 1) * 320].rearrange(
                        "m (d i) -> m d i", d=D, i=32)[:, :, :16],
                    s_sb[:, bt * 160:(bt + 1) * 160].rearrange(
                        "m (d i) -> m d i", d=D, i=16))
                for rnd, (dlo, nd) in enumerate(((0, 3), (3, 3), (6, 3),
                                                 (9, 1))):
                    ptv = psV.tile([128, 128], BF16, tag="vt")
                    nc.tensor.transpose(
                        ptv[:32 * nd, :],
                        s_bf[:, bt * 320 + dlo * 32:
                             bt * 320 + (dlo + nd) * 32],
                        ident[:, :])
                    nc.vector.tensor_copy(
                        vtd[:32 * nd, rnd * 256 + bt * 128:
                            rnd * 256 + bt * 128 + 128],
                        ptv[:32 * nd, :])

            def vt_ap(d, bt):
                return vtd[32 * (d % 3):32 * (d % 3) + 16,
                           (d // 3) * 256 + bt * 128:
                           (d // 3) * 256 + bt * 128 + 128]

            # per-digit g-phase mode: ACT drain + DVE mult, ACT drain +
            # Pool mult, or DVE mult reading PSUM f32 directly (1x)
            G_MODES = (["ad"] * (D - POOL_G_D - DVE_DIRECT_D)
                       + ["dv"] * DVE_DIRECT_D + ["ap"] * POOL_G_D)

            def g_phase(bt):
                # G[b, (d,p)] = sum_j x * (W @ s^T)   (raw, no alpha)
                for d in range(D):
                    mode = G_MODES[d]
                    eng = nc.gpsimd if mode == "ap" else nc.vector
                    yb = Y[:, (d % 2) * JP:(d % 2) * JP + JP]
                    ywb = Yw[:, (d % 2) * JP:(d % 2) * JP + JP]
                    for h in range(3):
                        pw = psW.tile([128, 384], F32, tag="wv")
                        q = 32 * (d % 3)
                        nc.tensor.matmul(
                            pw[:, :], vt_ap(d, bt),
                            wg[q:q + 16,
                               d * JP + h * 384:d * JP + (h + 1) * 384],
                            start=True, stop=True)
                        xs = xbp[:, bt * JP + h * 384:bt * JP + (h + 1) * 384]
                        if mode == "dv":
                            nc.vector.tensor_tensor(
                                yb[:, h * 384:(h + 1) * 384], pw[:, :], xs,
                                MULT)
                        else:
                            nc.scalar.copy(ywb[:, h * 384:(h + 1) * 384],
                                           pw[:, :])
                            eng.tensor_tensor(
                                yb[:, h * 384:(h + 1) * 384],
                                ywb[:, h * 384:(h + 1) * 384], xs, MULT)
                    eng.tensor_tensor(yb[:, 0:576], yb[:, 0:576],
                                      yb[:, 576:1152], ADD)
                    eng.tensor_tensor(yb[:, 0:288], yb[:, 0:288],
                                      yb[:, 288:576], ADD)
                    eng.tensor_tensor(
                        G[:, bt * 1440 + d * 144:bt * 1440 + (d + 1) * 144],
                        yb[:, 0:144], yb[:, 144:288], ADD)

            def e_update(r, bt):
                gb = G[:, bt * 1440:(bt + 1) * 1440]
                eb = E[:, bt * 1440:(bt + 1) * 1440]
                if r == 0:
                    nc.scalar.activation(eb, gb, AF.Exp, scale=alpha_bc[:, :1])
                else:
                    etb = Et[:, bt * 1440:(bt + 1) * 1440]
                    nc.scalar.activation(etb, gb, AF.Exp,
                                         scale=alpha_bc[:, :1])
                    nc.vector.tensor_tensor(eb, eb, etb, MULT)

            def softmax_c(bt):
                eb = E[:, bt * 1440:(bt + 1) * 1440]
                zb = zs[:, bt * 720:(bt + 1) * 720]
                nc.vector.tensor_tensor(zb[:, 0:720], eb[:, 0:720],
                                        eb[:, 720:1440], ADD)
                nc.vector.tensor_tensor(zb[:, 0:288], zb[:, 0:288],
                                        zb[:, 288:576], ADD)
                nc.vector.tensor_tensor(zb[:, 0:144], zb[:, 0:144],
                                        zb[:, 144:288], ADD)
                nc.vector.tensor_tensor(zb[:, 0:144], zb[:, 0:144],
                                        zb[:, 576:720], ADD)
                nc.vector.reciprocal(zrec[:, bt * 144:(bt + 1) * 144],
                                     zb[:, 0:144])
                nc.vector.tensor_copy(zrecb[:, bt * 144:(bt + 1) * 144],
                                      zrec[:, bt * 144:(bt + 1) * 144])
                nc.vector.tensor_tensor(
                    cb[:, bt * 1440:(bt + 1) * 1440].rearrange(
                        "m (d p) -> m d p", d=D, p=PL),
                    eb.rearrange("m (d p) -> m d p", d=D, p=PL),
                    zrecb[:, None, bt * 144:(bt + 1) * 144].to_broadcast(
                        (128, D, PL)),
                    MULT)

            # ---------------- schedule ----------------
            # Emission order == per-engine execution order, so each round
            # is emitted in dependency-topological order: bt0's post-
            # AllReduce work (vtd, raw g) is queued before anything that
            # waits on bt1's AllReduce (n2/alpha), and the next round's
            # bt0 softmax/s-contract runs while bt1's g-phase finishes.
            for r in range(NROUT_RUN):
                if r > 0:
                    softmax_c(0)
                s_contract(r, 0)
                bounce_bt(0)
                if r > 0:
                    softmax_c(1)
                ps = s_contract(r, 1)
                bounce_bt(1)
                last = (r == NROUT_RUN - 1)
                n2_partial(0)
                if not last:
                    vtd_path(0)
                    g_phase(0)              # waits only on AllReduce(bt0)
                    vtd_path(1)
                n2_partial(1)
                alpha_final(ps)
                if not last:
                    e_update(r, 0)          # exp: needs alpha + G(bt0)
                    g_phase(1)
                    e_update(r, 1)

            # ---- v = alpha * s ; layout already matches v_out ----
            for bt in range(2):
                nc.vector.tensor_scalar(sv[:, bt * 160:(bt + 1) * 160],
                                        s_sb[:, bt * 160:(bt + 1) * 160],
                                        alpha_bc[:, :1], None, MULT)
                nc.sync.dma_start(v_d.ap()[bt * 128:(bt + 1) * 128, :, :],
                                  sv[:, bt * 160:(bt + 1) * 160])

    nc.compile()
    return nc


def prep_inputs(primary_caps: np.ndarray, W: np.ndarray):
    """Host-side shard + layout prep. Returns in_maps for the 8 cores."""
    x = np.asarray(primary_caps, dtype=np.float32)
    Wf = np.asarray(W, dtype=np.float32)
    bf = ml_dtypes.bfloat16
    ident = np.eye(128, dtype=np.float32).astype(bf)
    in_maps = []
    for kcore in range(CORES):
        pk = slice(kcore * PL, (kcore + 1) * PL)
        xk = x[:, pk, :]                       # [256, 144, 8]
        Wk = Wf[:, pk, :, :]                   # [10, 144, 16, 8]

        # x_bp [128, 2*1152]: row b%128, col (bt*1152 + j*144 + p)
        xbp = xk.transpose(0, 2, 1).reshape(B, JP)
        xbp = xbp.reshape(2, 128, JP).transpose(1, 0, 2).reshape(128, 2 * JP)

        # x_t9 [128, 9*256]: 0.1*x^T: row (j*144+p)%128, col k*256 + b
        xt9 = 0.1 * xk.transpose(2, 1, 0).reshape(JP, B)
        xt9 = xt9.reshape(NK, 128, B).transpose(1, 0, 2).reshape(128, NK * B)

        # w_sc [128, 10*9*16]: row (j*144+p)%128, col (d*9+k)*16 + i
        wsc = Wk.transpose(0, 3, 1, 2).reshape(D, JP, I)
        wsc = wsc.reshape(D, NK, 128, I).transpose(2, 0, 1, 3)
        wsc = wsc.reshape(128, D * NK * I)

        # w_g [128, D*J*PL]: rows 32q+i (4 replicas), col d*1152 + j*144 + p
        wg1 = Wk.transpose(2, 0, 3, 1).reshape(I, D * JP)
        wg = np.zeros((128, D * JP), dtype=np.float32)
        for q in range(3):
            wg[32 * q:32 * q + I] = wg1

        in_maps.append({
            "x_bp": xbp.astype(bf),
            "x_t9": xt9.astype(bf),
            "w_sc": wsc.astype(bf),
            "w_g": wg.astype(bf),
            "ident": ident,
        })
    return in_maps


_NC_CACHE = None


def get_program():
    global _NC_CACHE
    if _NC_CACHE is None:
        _NC_CACHE = build_program()
    return _NC_CACHE


def kernel(primary_caps: np.ndarray, W: np.ndarray) -> np.ndarray:
    nc = get_program()
    in_maps = prep_inputs(primary_caps, W)
    res = run_bass_kernel_spmd(nc, in_maps, core_ids=list(range(CORES)))
    return np.asarray(res.results[0]["v_out"], dtype=np.float32)


if __name__ == "__main__":
    rng = np.random.default_rng(0)
    x = rng.standard_normal((B, P, J), dtype=np.float32)
    W = rng.standard_normal((D, P, I, J), dtype=np.float32).astype(np.float32)
    out = kernel(x, W)
    print("out", out.shape, out.dtype, float(np.abs(out).mean()))


# revision 20
# speedup vs baseline: 1.5105x; 1.0459x over previous
"""CapsNet DigitCaps routing kernel for 8 TRN2 NeuronCores — v2.

Strategy: shard the 1152 primary capsules across the 8 cores (144 each),
keep the full batch (256, as two 128-row halves bt) on every core.

v2 never materializes u_hat. Per routing iteration:

  s-phase:   s[b,d,i] = sum_{p,j} W[d,p,i,j] * (c[b,d,p] * x[b,p,j])
    xc = c (*) x is a single broadcast multiply in j-space (8 wide, half
    the elements of c*u_hat); xc is PE-transposed per digit into (j,p)-
    partition tiles, drained to SBUF, and contracted with a host-packed
    W operand on the TensorEngine (9 accumulating K=128, N=16 matmuls
    per (bt,d)) straight into s[b, (d,i)] layout.
    At r=0, c == 0.1 uniformly, so the transposed x (times 0.1) is a
    static input and the s-phase is matmuls only.

  squash:    global-Frobenius-norm alpha; the per-core partial s is
    AllReduced (per-bt round trips so bt0's collective overlaps bt1's
    compute), then every core computes the same
    alpha = n2 / ((n2+1)(sqrt(n2)+eps)).

  g-phase (j-space): wv[b,d,p,j] = sum_i W[d,p,i,j] * s[b,d,i] on the
    TensorEngine (lhsT = s^T tiles from a post-AllReduce PE transpose),
    output kept bf16 in PSUM; g_raw = sum_j x (*) wv via a DVE/Pool
    multiply that reads PSUM directly plus a j-tree. alpha is never
    applied to wv or g_raw: the routing state is kept as
    E = prod_r exp(alpha_r * g_raw_r), updated with exp(scale=alpha)
    on the Activation engine and one bf16 multiply — softmax c is then
    E * (1/sum_d E).

Layouts (per core, SBUF partition dim first, p = local primary index):
  xbp [128, 2*1152] bf16   col = bt*1152 + j*144 + p      (g multiply)
  xt9 [128, 9*256]  bf16   0.1*x^T: row = (j*144+p)%128, col = k*256+b
  wsc [128, 1440]   bf16   row = (j*144+p)%128, col = (d*9+k)*16 + i
  wg  [16, 11520]   bf16   row = i, col = d*1152 + j*144 + p
  s_sb [128, 2*160] f32    col = bt*160 + d*16 + i  (matches v_out!)
  E,G,c [128, *1440] bf16  col = d*144 + p (d-major)
  xc  [128, 11520]  bf16   col = d*1152 + j*144 + p (per bt)
  xcT [128, 11520]  bf16   col = (d*9+k)*128 + (jp%128), value xc^T
  vtd [16, 2560]    bf16   col = d*256 + bt*128 + b  (s^T for wv lhsT)
"""

import os
import sys

for _p in ("/opt/trn_rl_repo",):
    if _p not in sys.path and os.path.isdir(_p):
        sys.path.insert(0, _p)

import numpy as np
import ml_dtypes

import concourse.bass as bass
import concourse.bacc as bacc
import concourse.mybir as mybir
import concourse.tile as tile
from concourse.bass_utils import run_bass_kernel_spmd

F32 = mybir.dt.float32
BF16 = mybir.dt.bfloat16
MULT = mybir.AluOpType.mult
ADD = mybir.AluOpType.add
AF = mybir.ActivationFunctionType

B, D, P, I, J = 256, 10, 1152, 16, 8
CORES = 8
PL = P // CORES          # 144 local primary capsules
JP = J * PL              # 1152 (j,p) rows per digit
NK = JP // 128           # 9 partition chunks per digit
EPS = 1e-7
NROUT = 3

# routing rounds actually executed (debug bisection: 1, 2, or 3)
STAGE = int(os.environ.get("BASSCAPS_STAGE", "3"))
# Replace the AllReduce with a local DRAM copy (for TimelineSim profiling).
NO_CC = os.environ.get("BASSCAPS_NO_CC", "0") == "1"
# Pool engine shares
POOL_XC_D = int(os.environ.get("BASSCAPS_POOL_XC_D", "2"))   # digits of xc mult
POOL_G_D = int(os.environ.get("BASSCAPS_POOL_G_D", "2"))     # digits of g phase
DVE_DIRECT_D = int(os.environ.get("BASSCAPS_DVE_DIRECT_D", "1"))
# of the 12 xcT drain batches per bt: how many go to ACT / Pool (rest DVE)
DRAIN_ACT = int(os.environ.get("BASSCAPS_DRAIN_ACT", "6"))


def build_program():
    nc = bacc.Bacc("TRN2", target_bir_lowering=False, debug=False,
                   num_devices=CORES)

    xbp_d = nc.dram_tensor("x_bp", [128, 2 * JP], BF16, kind="ExternalInput")
    xt9_d = nc.dram_tensor("x_t9", [128, NK * 256], BF16, kind="ExternalInput")
    wsc_d = nc.dram_tensor("w_sc", [128, D * NK * 16], BF16,
                           kind="ExternalInput")
    wg_d = nc.dram_tensor("w_g", [128, D * JP], BF16, kind="ExternalInput")
    ident_d = nc.dram_tensor("ident", [128, 128], BF16, kind="ExternalInput")
    v_d = nc.dram_tensor("v_out", [B, D, I], F32, kind="ExternalOutput")

    NROUT_RUN = max(1, min(NROUT, STAGE))

    with tile.TileContext(nc) as tc:
        with (
            tc.tile_pool(name="persist", bufs=1) as pp,
            tc.tile_pool(name="psS", bufs=2, space=bass.MemorySpace.PSUM) as psS,
            tc.tile_pool(name="psW", bufs=5, space=bass.MemorySpace.PSUM) as psW,
            tc.tile_pool(name="psV", bufs=1, space=bass.MemorySpace.PSUM) as psV,
            tc.tile_pool(name="dram", bufs=1, space=bass.MemorySpace.DRAM) as dp,
        ):
            xbp = pp.tile([128, 2 * JP], BF16)
            xt9 = pp.tile([128, NK * 256], BF16)
            wsc = pp.tile([128, D * NK * 16], BF16)
            wg = pp.tile([128, D * JP], BF16)
            ident = pp.tile([128, 128], BF16)

            s_sb = pp.tile([128, 2 * 160], F32)
            sv = pp.tile([128, 2 * 160], F32)
            s_bf = pp.tile([128, 2 * 320], BF16)   # pad-32 staging for vt
            vtd = pp.tile([128, 4 * 256], BF16)
            E = pp.tile([128, 2 * 1440], BF16)
            Et = pp.tile([128, 2 * 1440], BF16)
            G = pp.tile([128, 2 * 1440], BF16)
            cb = pp.tile([128, 2 * 1440], BF16)
            zs = pp.tile([128, 2 * 720], BF16)
            zrec = pp.tile([128, 2 * 144], F32)
            zrecb = pp.tile([128, 2 * 144], BF16)
            xc = pp.tile([128, 2 * D * JP], BF16)   # per-bt halves
            xcT = pp.tile([128, 2 * D * JP], BF16)
            Y = pp.tile([128, 3 * JP], BF16)       # g-phase scratch (3 bufs)
            Yw = pp.tile([128, 3 * JP], BF16)      # wv drain staging

            sq = pp.tile([128, 2 * 160], F32)
            accb = pp.tile([128, 2], F32)
            acc = pp.tile([128, 1], F32)
            ones = pp.tile([128, 128], F32)
            n2sb = pp.tile([128, 1], F32)
            t1 = pp.tile([128, 1], F32)
            r1 = pp.tile([128, 1], F32)
            lnv = pp.tile([128, 1], F32)
            rt = pp.tile([128, 1], F32)
            t2 = pp.tile([128, 1], F32)
            r2 = pp.tile([128, 1], F32)
            alpha_bc = pp.tile([128, 1], F32)

            bounce_in = dp.tile([B, 160], F32)
            bounce_out = dp.tile([B, 160], F32)

            nc.vector.memset(ones[:, :], 1.0)
            # warm the ACT exp/ln table set
            nc.scalar.activation(t1[:, :], ones[:, :1], AF.Exp)
            nc.scalar.activation(t2[:, :], ones[:, :1], AF.Ln)

            nc.sync.dma_start(xt9[:, :], xt9_d.ap())
            nc.sync.dma_start(wsc[:, :], wsc_d.ap())
            nc.sync.dma_start(ident[:, :], ident_d.ap())
            nc.sync.dma_start(xbp[:, :], xbp_d.ap())
            nc.sync.dma_start(wg[:, :], wg_d.ap())

            # ---------------- helpers ----------------

            def xc_chunk(bt, dlo, dn, eng):
                """xc[:, dlo:dlo+dn digits] = c (*) x for batch-half bt."""
                xc4 = xc[:, bt * D * JP:(bt + 1) * D * JP].rearrange(
                    "m (d j p) -> m d j p", d=D, j=J, p=PL)
                cv = cb[:, bt * 1440:(bt + 1) * 1440].rearrange(
                    "m (d p) -> m d p", d=D, p=PL)
                xv = xbp[:, bt * JP:(bt + 1) * JP].rearrange(
                    "m (j p) -> m j p", j=J, p=PL)
                eng.tensor_tensor(
                    xc4[:, dlo:dlo + dn],
                    cv[:, dlo:dlo + dn, None, :].to_broadcast(
                        (128, dn, J, PL)),
                    xv[:, None, :, :].to_broadcast((128, dn, J, PL)),
                    MULT)

            def s_matmuls(r, bt, ps, dlo, dn):
                for d in range(dlo, dlo + dn):
                    for k in range(NK):
                        if r == 0:
                            lhsT = xt9[:, k * 256 + bt * 128:
                                       k * 256 + bt * 128 + 128]
                        else:
                            lhsT = xcT[:, (bt * D + d) * JP + k * 128:
                                       (bt * D + d) * JP + (k + 1) * 128]
                        nc.tensor.matmul(
                            ps[:, d * 16:(d + 1) * 16],
                            lhsT,
                            wsc[:, (d * NK + k) * 16:(d * NK + k + 1) * 16],
                            start=(k == 0), stop=(k == NK - 1))

            # xc digit groups; third group's multiply goes to Pool
            XCG = ((0, 3, nc.vector), (3, 3, nc.vector),
                   (6, 2, nc.gpsimd), (8, 2, nc.vector))

            def s_phase_front(r, bt):
                """softmax -> xc multiply -> xbar transpose for half bt.

                dma_start_transpose semantics: out[r, k, b] = in[b,
                128k + r], which lands xc^T exactly in the (d,k)-chunk
                layout the contraction matmuls want.
                """
                if r > 0:
                    softmax_c(bt)
                    for glo, gn, geng in XCG:
                        xc_chunk(bt, glo, gn, geng)
                        base = bt * D * JP
                        nc.sync.dma_start_transpose(
                            xcT[:, base + glo * JP:
                                base + (glo + gn) * JP].rearrange(
                                "m (k b) -> m k b", k=gn * NK, b=128),
                            xc[:, base + glo * JP:base + (glo + gn) * JP])

            def s_phase_mm(r, bt):
                """contraction matmuls + s drain + AllReduce kick."""
                ps = psS.tile([128, 176], F32, tag="s")
                for glo, gn, _ in XCG:
                    s_matmuls(r, bt, ps, glo, gn)
                nc.vector.tensor_copy(s_sb[:, bt * 160:(bt + 1) * 160],
                                      ps[:, :160])
                bounce_bt(bt)
                return ps

            def bounce_bt(bt):
                nc.sync.dma_start(
                    bounce_in[bt * 128:(bt + 1) * 128, :],
                    s_sb[:, bt * 160:(bt + 1) * 160])
                if NO_CC:
                    nc.sync.dma_start(
                        bounce_out[bt * 128:(bt + 1) * 128, :],
                        bounce_in[bt * 128:(bt + 1) * 128, :])
                else:
                    nc.gpsimd.collective_compute(
                        "AllReduce", ADD,
                        ins=[bounce_in[bt * 128:(bt + 1) * 128, :].opt()],
                        outs=[bounce_out[bt * 128:(bt + 1) * 128, :].opt()],
                        replica_groups=[list(range(CORES))],
                    )
                nc.sync.dma_start(
                    s_sb[:, bt * 160:(bt + 1) * 160],
                    bounce_out[bt * 128:(bt + 1) * 128, :])

            def n2_partial(bt):
                ss = s_sb[:, bt * 160:(bt + 1) * 160]
                nc.vector.tensor_tensor(sq[:, bt * 160:(bt + 1) * 160],
                                        ss, ss, MULT)
                nc.vector.tensor_reduce(
                    accb[:, bt:bt + 1],
                    sq[:, None, bt * 160:(bt + 1) * 160],
                    mybir.AxisListType.X, ADD)

            def alpha_final(psn):
                # alpha = n2 / ((n2+1)(sqrt(n2)+eps)); sqrt via ln/exp so
                # ACT stays on the exp/ln table set. psn: spare cols
                # [160:161] of the last s-contract PSUM tile.
                nc.vector.tensor_tensor(acc[:, :], accb[:, 0:1], accb[:, 1:2],
                                        ADD)
                nc.tensor.matmul(psn[:, 160:161], ones[:, :], acc[:, :],
                                 start=True, stop=True)
                nc.vector.tensor_copy(n2sb[:, :], psn[:, 160:161])
                nc.vector.tensor_scalar_add(t1[:, :], n2sb[:, :], 1.0)
                nc.vector.reciprocal(r1[:, :], t1[:, :])
                nc.scalar.activation(lnv[:, :], n2sb[:, :], AF.Ln)
                nc.scalar.activation(rt[:, :], lnv[:, :], AF.Exp, scale=0.5)
                nc.vector.tensor_scalar_add(t2[:, :], rt[:, :], EPS)
                nc.vector.reciprocal(r2[:, :], t2[:, :])
                nc.vector.tensor_tensor(alpha_bc[:, :], n2sb[:, :], r1[:, :],
                                        MULT)
                nc.vector.tensor_tensor(alpha_bc[:, :], alpha_bc[:, :],
                                        r2[:, :], MULT)

            def vtd_path(bt):
                # s^T tiles for the wv matmul lhsT (post-AllReduce s).
                # vtd row layout: digit d lives at partitions 32*(d%4)..+16
                # of round d//4; lhsT base partitions are 0/32/64/96.
                nc.vector.tensor_copy(
                    s_bf[:, bt * 320:(bt + 1) * 320].rearrange(
                        "m (d i) -> m d i", d=D, i=32)[:, :, :16],
                    s_sb[:, bt * 160:(bt + 1) * 160].rearrange(
                        "m (d i) -> m d i", d=D, i=16))
                for rnd, (dlo, nd) in enumerate(((0, 3), (3, 3), (6, 3),
                                                 (9, 1))):
                    ptv = psV.tile([128, 128], BF16, tag="vt")
                    nc.tensor.transpose(
                        ptv[:32 * nd, :],
                        s_bf[:, bt * 320 + dlo * 32:
                             bt * 320 + (dlo + nd) * 32],
                        ident[:, :])
                    nc.vector.tensor_copy(
                        vtd[:32 * nd, rnd * 256 + bt * 128:
                            rnd * 256 + bt * 128 + 128],
                        ptv[:32 * nd, :])

            def vt_ap(d, bt):
                return vtd[32 * (d % 3):32 * (d % 3) + 16,
                           (d // 3) * 256 + bt * 128:
                           (d // 3) * 256 + bt * 128 + 128]

            # per-digit g-phase mode: ACT drain + DVE mult, ACT drain +
            # Pool mult, or DVE mult reading PSUM f32 directly (1x)
            G_MODES = (["ad"] * (D - POOL_G_D - DVE_DIRECT_D)
                       + ["dv"] * DVE_DIRECT_D + ["ap"] * POOL_G_D)

            def g_phase(bt):
                # G[b, (d,p)] = sum_j x * (W @ s^T)   (raw, no alpha)
                for d in range(D):
                    mode = G_MODES[d]
                    eng = nc.gpsimd if mode == "ap" else nc.vector
                    yb = Y[:, (d % 3) * JP:(d % 3) * JP + JP]
                    ywb = Yw[:, (d % 3) * JP:(d % 3) * JP + JP]
                    for h in range(3):
                        pw = psW.tile([128, 384], F32, tag="wv")
                        q = 32 * (d % 3)
                        nc.tensor.matmul(
                            pw[:, :], vt_ap(d, bt),
                            wg[q:q + 16,
                               d * JP + h * 384:d * JP + (h + 1) * 384],
                            start=True, stop=True)
                        xs = xbp[:, bt * JP + h * 384:bt * JP + (h + 1) * 384]
                        if mode == "dv":
                            nc.vector.tensor_tensor(
                                yb[:, h * 384:(h + 1) * 384], pw[:, :], xs,
                                MULT)
                        else:
                            nc.scalar.copy(ywb[:, h * 384:(h + 1) * 384],
                                           pw[:, :])
                            eng.tensor_tensor(
                                yb[:, h * 384:(h + 1) * 384],
                                ywb[:, h * 384:(h + 1) * 384], xs, MULT)
                    eng.tensor_tensor(yb[:, 0:576], yb[:, 0:576],
                                      yb[:, 576:1152], ADD)
                    eng.tensor_tensor(yb[:, 0:288], yb[:, 0:288],
                                      yb[:, 288:576], ADD)
                    eng.tensor_tensor(
                        G[:, bt * 1440 + d * 144:bt * 1440 + (d + 1) * 144],
                        yb[:, 0:144], yb[:, 144:288], ADD)

            def e_update(r, bt):
                gb = G[:, bt * 1440:(bt + 1) * 1440]
                eb = E[:, bt * 1440:(bt + 1) * 1440]
                if r == 0:
                    nc.scalar.activation(eb, gb, AF.Exp, scale=alpha_bc[:, :1])
                else:
                    etb = Et[:, bt * 1440:(bt + 1) * 1440]
                    nc.scalar.activation(etb, gb, AF.Exp,
                                         scale=alpha_bc[:, :1])
                    nc.vector.tensor_tensor(eb, eb, etb, MULT)

            def softmax_c(bt):
                eb = E[:, bt * 1440:(bt + 1) * 1440]
                zb = zs[:, bt * 720:(bt + 1) * 720]
                nc.vector.tensor_tensor(zb[:, 0:720], eb[:, 0:720],
                                        eb[:, 720:1440], ADD)
                nc.vector.tensor_tensor(zb[:, 0:288], zb[:, 0:288],
                                        zb[:, 288:576], ADD)
                nc.vector.tensor_tensor(zb[:, 0:144], zb[:, 0:144],
                                        zb[:, 144:288], ADD)
                nc.vector.tensor_tensor(zb[:, 0:144], zb[:, 0:144],
                                        zb[:, 576:720], ADD)
                nc.vector.reciprocal(zrec[:, bt * 144:(bt + 1) * 144],
                                     zb[:, 0:144])
                nc.vector.tensor_copy(zrecb[:, bt * 144:(bt + 1) * 144],
                                      zrec[:, bt * 144:(bt + 1) * 144])
                nc.vector.tensor_tensor(
                    cb[:, bt * 1440:(bt + 1) * 1440].rearrange(
                        "m (d p) -> m d p", d=D, p=PL),
                    eb.rearrange("m (d p) -> m d p", d=D, p=PL),
                    zrecb[:, None, bt * 144:(bt + 1) * 144].to_broadcast(
                        (128, D, PL)),
                    MULT)

            # ---------------- schedule ----------------
            # Emission order == per-engine execution order. Rounds are
            # software-pipelined: the next round's bt0 softmax/xc/xbar
            # front is emitted between this round's two g-phases.
            for r in range(NROUT_RUN):
                if r == 0:
                    s_phase_front(0, 0)
                    s_phase_front(0, 1)
                s_phase_mm(r, 0)
                if r == 0:
                    s_phase_front(r, 1)
                ps = s_phase_mm(r, 1)
                last = (r == NROUT_RUN - 1)
                if last:
                    n2_partial(0)
                    n2_partial(1)
                    alpha_final(ps)
                else:
                    vtd_path(0)
                    g_phase(0)              # waits only on AllReduce(bt0)
                    vtd_path(1)
                    n2_partial(0)
                    n2_partial(1)
                    alpha_final(ps)
                    e_update(r, 0)
                    s_phase_front(r + 1, 0)
                    g_phase(1)
                    e_update(r, 1)
                    s_phase_front(r + 1, 1)

            # ---- v = alpha * s ; layout already matches v_out ----
            for bt in range(2):
                nc.vector.tensor_scalar(sv[:, bt * 160:(bt + 1) * 160],
                                        s_sb[:, bt * 160:(bt + 1) * 160],
                                        alpha_bc[:, :1], None, MULT)
                nc.sync.dma_start(v_d.ap()[bt * 128:(bt + 1) * 128, :, :],
                                  sv[:, bt * 160:(bt + 1) * 160])

    nc.compile()
    return nc


def prep_inputs(primary_caps: np.ndarray, W: np.ndarray):
    """Host-side shard + layout prep. Returns in_maps for the 8 cores."""
    x = np.asarray(primary_caps, dtype=np.float32)
    Wf = np.asarray(W, dtype=np.float32)
    bf = ml_dtypes.bfloat16
    ident = np.eye(128, dtype=np.float32).astype(bf)
    in_maps = []
    for kcore in range(CORES):
        pk = slice(kcore * PL, (kcore + 1) * PL)
        xk = x[:, pk, :]                       # [256, 144, 8]
        Wk = Wf[:, pk, :, :]                   # [10, 144, 16, 8]

        # x_bp [128, 2*1152]: row b%128, col (bt*1152 + j*144 + p)
        xbp = xk.transpose(0, 2, 1).reshape(B, JP)
        xbp = xbp.reshape(2, 128, JP).transpose(1, 0, 2).reshape(128, 2 * JP)

        # x_t9 [128, 9*256]: 0.1*x^T: row (j*144+p)%128, col k*256 + b
        xt9 = 0.1 * xk.transpose(2, 1, 0).reshape(JP, B)
        xt9 = xt9.reshape(NK, 128, B).transpose(1, 0, 2).reshape(128, NK * B)

        # w_sc [128, 10*9*16]: row (j*144+p)%128, col (d*9+k)*16 + i
        wsc = Wk.transpose(0, 3, 1, 2).reshape(D, JP, I)
        wsc = wsc.reshape(D, NK, 128, I).transpose(2, 0, 1, 3)
        wsc = wsc.reshape(128, D * NK * I)

        # w_g [128, D*J*PL]: rows 32q+i (4 replicas), col d*1152 + j*144 + p
        wg1 = Wk.transpose(2, 0, 3, 1).reshape(I, D * JP)
        wg = np.zeros((128, D * JP), dtype=np.float32)
        for q in range(3):
            wg[32 * q:32 * q + I] = wg1

        in_maps.append({
            "x_bp": xbp.astype(bf),
            "x_t9": xt9.astype(bf),
            "w_sc": wsc.astype(bf),
            "w_g": wg.astype(bf),
            "ident": ident,
        })
    return in_maps


_NC_CACHE = None


def get_program():
    global _NC_CACHE
    if _NC_CACHE is None:
        _NC_CACHE = build_program()
    return _NC_CACHE


def kernel(primary_caps: np.ndarray, W: np.ndarray) -> np.ndarray:
    nc = get_program()
    in_maps = prep_inputs(primary_caps, W)
    res = run_bass_kernel_spmd(nc, in_maps, core_ids=list(range(CORES)))
    return np.asarray(res.results[0]["v_out"], dtype=np.float32)


if __name__ == "__main__":
    rng = np.random.default_rng(0)
    x = rng.standard_normal((B, P, J), dtype=np.float32)
    W = rng.standard_normal((D, P, I, J), dtype=np.float32).astype(np.float32)
    out = kernel(x, W)
    print("out", out.shape, out.dtype, float(np.abs(out).mean()))


# revision 34
# speedup vs baseline: 1.7664x; 1.1694x over previous
"""CapsNet DigitCaps routing kernel for 8 TRN2 NeuronCores — v2.

Strategy: shard the 1152 primary capsules across the 8 cores (144 each),
keep the full batch (256, as two 128-row halves bt) on every core.

v2 never materializes u_hat. Per routing iteration:

  s-phase:   s[b,d,i] = sum_{p,j} W[d,p,i,j] * (c[b,d,p] * x[b,p,j])
    xc = c (*) x is a single broadcast multiply in j-space (8 wide, half
    the elements of c*u_hat); xc is PE-transposed per digit into (j,p)-
    partition tiles, drained to SBUF, and contracted with a host-packed
    W operand on the TensorEngine (9 accumulating K=128, N=16 matmuls
    per (bt,d)) straight into s[b, (d,i)] layout.
    At r=0, c == 0.1 uniformly, so the transposed x (times 0.1) is a
    static input and the s-phase is matmuls only.

  squash:    global-Frobenius-norm alpha; the per-core partial s is
    AllReduced (per-bt round trips so bt0's collective overlaps bt1's
    compute), then every core computes the same
    alpha = n2 / ((n2+1)(sqrt(n2)+eps)).

  g-phase (j-space): wv[b,d,p,j] = sum_i W[d,p,i,j] * s[b,d,i] on the
    TensorEngine (lhsT = s^T tiles from a post-AllReduce PE transpose),
    output kept bf16 in PSUM; g_raw = sum_j x (*) wv via a DVE/Pool
    multiply that reads PSUM directly plus a j-tree. alpha is never
    applied to wv or g_raw: the routing state is kept as
    E = prod_r exp(alpha_r * g_raw_r), updated with exp(scale=alpha)
    on the Activation engine and one bf16 multiply — softmax c is then
    E * (1/sum_d E).

Layouts (per core, SBUF partition dim first, p = local primary index):
  xbp [128, 2*1152] bf16   col = bt*1152 + j*144 + p      (g multiply)
  xt9 [128, 9*256]  bf16   0.1*x^T: row = (j*144+p)%128, col = k*256+b
  wsc [128, 1440]   bf16   row = (j*144+p)%128, col = (d*9+k)*16 + i
  wg  [16, 11520]   bf16   row = i, col = d*1152 + j*144 + p
  s_sb [128, 2*160] f32    col = bt*160 + d*16 + i  (matches v_out!)
  E,G,c [128, *1440] bf16  col = d*144 + p (d-major)
  xc  [128, 11520]  bf16   col = d*1152 + j*144 + p (per bt)
  xcT [128, 11520]  bf16   col = (d*9+k)*128 + (jp%128), value xc^T
  vtd [16, 2560]    bf16   col = d*256 + bt*128 + b  (s^T for wv lhsT)
"""

import os
import sys

for _p in ("/opt/trn_rl_repo",):
    if _p not in sys.path and os.path.isdir(_p):
        sys.path.insert(0, _p)

import numpy as np
import ml_dtypes

import concourse.bass as bass
import concourse.bacc as bacc
import concourse.mybir as mybir
import concourse.tile as tile
from concourse.bass_utils import run_bass_kernel_spmd

F32 = mybir.dt.float32
BF16 = mybir.dt.bfloat16
MULT = mybir.AluOpType.mult
ADD = mybir.AluOpType.add
AF = mybir.ActivationFunctionType

B, D, P, I, J = 256, 10, 1152, 16, 8
CORES = 8
PL = P // CORES          # 144 local primary capsules
JP = J * PL              # 1152 (j,p) rows per digit
NK = JP // 128           # 9 partition chunks per digit
EPS = 1e-7
NROUT = 3

# routing rounds actually executed (debug bisection: 1, 2, or 3)
STAGE = int(os.environ.get("BASSCAPS_STAGE", "3"))
# Replace the AllReduce with a local DRAM copy (for TimelineSim profiling).
NO_CC = os.environ.get("BASSCAPS_NO_CC", "0") == "1"
# Pool engine shares
POOL_XC_D = int(os.environ.get("BASSCAPS_POOL_XC_D", "2"))   # digits of xc mult
POOL_G_D = int(os.environ.get("BASSCAPS_POOL_G_D", "2"))     # digits of g phase
DVE_DIRECT_D = int(os.environ.get("BASSCAPS_DVE_DIRECT_D", "1"))
# of the 12 xcT drain batches per bt: how many go to ACT / Pool (rest DVE)
DRAIN_ACT = int(os.environ.get("BASSCAPS_DRAIN_ACT", "6"))


def build_program():
    nc = bacc.Bacc("TRN2", target_bir_lowering=False, debug=False,
                   num_devices=CORES)

    xbp_d = nc.dram_tensor("x_bp", [128, 2 * JP], BF16, kind="ExternalInput")
    xt9_d = nc.dram_tensor("x_t9", [128, NK * 256], BF16, kind="ExternalInput")
    wsc_d = nc.dram_tensor("w_sc", [128, D * NK * 16], BF16,
                           kind="ExternalInput")
    wg_d = nc.dram_tensor("w_g", [128, D * JP], BF16, kind="ExternalInput")
    ident_d = nc.dram_tensor("ident", [128, 128], BF16, kind="ExternalInput")
    v_d = nc.dram_tensor("v_out", [B, D, I], F32, kind="ExternalOutput")

    NROUT_RUN = max(1, min(NROUT, STAGE))

    with tile.TileContext(nc) as tc:
        with (
            tc.tile_pool(name="persist", bufs=1) as pp,
            tc.tile_pool(name="psS", bufs=2, space=bass.MemorySpace.PSUM) as psS,
            tc.tile_pool(name="psW", bufs=5, space=bass.MemorySpace.PSUM) as psW,
            tc.tile_pool(name="psV", bufs=1, space=bass.MemorySpace.PSUM) as psV,
            tc.tile_pool(name="dram", bufs=1, space=bass.MemorySpace.DRAM) as dp,
        ):
            xbp = pp.tile([128, 2 * JP], BF16)
            xt9 = pp.tile([128, NK * 256], BF16)
            wsc = pp.tile([128, D * NK * 16], BF16)
            wg = pp.tile([128, D * JP], BF16)
            ident = pp.tile([128, 128], BF16)

            s_sb = pp.tile([128, 2 * 160], F32)
            sbb = pp.tile([128, 2 * 160], BF16)    # bf16 AllReduce payload
            sv = pp.tile([128, 2 * 160], F32)
            s_bf = pp.tile([128, 2 * 320], BF16)   # pad-32 staging for vt
            vtd = pp.tile([128, 4 * 256], BF16)
            E = pp.tile([128, 2 * 1440], BF16)
            Et = pp.tile([128, 2 * 1440], BF16)
            G = pp.tile([128, 2 * 1440], BF16)
            cb = pp.tile([128, 2 * 1440], BF16)
            zs = pp.tile([128, 2 * 720], BF16)
            zrec = pp.tile([128, 2 * 144], F32)
            zrecb = pp.tile([128, 2 * 144], BF16)
            xc = pp.tile([128, 2 * D * JP], BF16)   # per-bt halves
            xcT = pp.tile([128, 2 * D * JP], BF16)
            Y = pp.tile([128, 3 * JP], BF16)       # g-phase scratch (3 bufs)
            Yw = pp.tile([128, 3 * JP], BF16)      # wv drain staging

            sq = pp.tile([128, 2 * 160], F32)
            accb = pp.tile([128, 2], F32)
            acc = pp.tile([128, 1], F32)
            ones = pp.tile([128, 128], F32)
            n2sb = pp.tile([128, 1], F32)
            t1 = pp.tile([128, 1], F32)
            r1 = pp.tile([128, 1], F32)
            lnv = pp.tile([128, 1], F32)
            rt = pp.tile([128, 1], F32)
            t2 = pp.tile([128, 1], F32)
            r2 = pp.tile([128, 1], F32)
            alpha_bc = pp.tile([128, 1], F32)

            bounce_in = dp.tile([B, 160], BF16)
            bounce_out = dp.tile([B, 160], BF16)

            nc.vector.memset(ones[:, :], 1.0)
            # warm the ACT exp/ln table set
            nc.scalar.activation(t1[:, :], ones[:, :1], AF.Exp)
            nc.scalar.activation(t2[:, :], ones[:, :1], AF.Ln)

            nc.sync.dma_start(xt9[:, :2 * 256], xt9_d.ap()[:, :2 * 256])
            nc.sync.dma_start(wsc[:, :], wsc_d.ap())
            nc.sync.dma_start(xt9[:, 2 * 256:], xt9_d.ap()[:, 2 * 256:])
            nc.sync.dma_start(ident[:, :], ident_d.ap())

            def load_rest():
                # emitted after the r0 AllReduce kick so the bounce DMAs
                # don't queue behind these big transfers
                nc.sync.dma_start(xbp[:, :], xbp_d.ap())
                for q in range(4):
                    w = D * JP // 4
                    nc.sync.dma_start(wg[:, q * w:(q + 1) * w],
                                      wg_d.ap()[:, q * w:(q + 1) * w])

            # ---------------- helpers ----------------

            def xc_chunk(bt, dlo, dn, eng):
                """xc[:, dlo:dlo+dn digits] = c (*) x for batch-half bt."""
                xc4 = xc[:, bt * D * JP:(bt + 1) * D * JP].rearrange(
                    "m (d j p) -> m d j p", d=D, j=J, p=PL)
                cv = cb[:, bt * 1440:(bt + 1) * 1440].rearrange(
                    "m (d p) -> m d p", d=D, p=PL)
                xv = xbp[:, bt * JP:(bt + 1) * JP].rearrange(
                    "m (j p) -> m j p", j=J, p=PL)
                eng.tensor_tensor(
                    xc4[:, dlo:dlo + dn],
                    cv[:, dlo:dlo + dn, None, :].to_broadcast(
                        (128, dn, J, PL)),
                    xv[:, None, :, :].to_broadcast((128, dn, J, PL)),
                    MULT)

            def s_matmuls(r, bt, ps, dlo, dn):
                for d in range(dlo, dlo + dn):
                    for k in range(NK):
                        if r == 0:
                            lhsT = xt9[:, k * 256 + bt * 128:
                                       k * 256 + bt * 128 + 128]
                        else:
                            lhsT = xcT[:, (bt * D + d) * JP + k * 128:
                                       (bt * D + d) * JP + (k + 1) * 128]
                        nc.tensor.matmul(
                            ps[:, d * 16:(d + 1) * 16],
                            lhsT,
                            wsc[:, (d * NK + k) * 16:(d * NK + k + 1) * 16],
                            start=(k == 0), stop=(k == NK - 1))

            # xc digit groups; third group's multiply goes to Pool
            XCG = ((0, 3, nc.vector), (3, 3, nc.vector),
                   (6, 2, nc.gpsimd), (8, 2, nc.vector))

            def s_phase_front(r, bt):
                """softmax -> xc multiply -> xbar transpose for half bt.

                dma_start_transpose semantics: out[r, k, b] = in[b,
                128k + r], which lands xc^T exactly in the (d,k)-chunk
                layout the contraction matmuls want.
                """
                if r > 0:
                    softmax_c(bt)
                    for glo, gn, geng in XCG:
                        xc_chunk(bt, glo, gn, geng)
                        base = bt * D * JP
                        nc.sync.dma_start_transpose(
                            xcT[:, base + glo * JP:
                                base + (glo + gn) * JP].rearrange(
                                "m (k b) -> m k b", k=gn * NK, b=128),
                            xc[:, base + glo * JP:base + (glo + gn) * JP])

            def s_phase_mm(r, bt):
                """contraction matmuls + s drain + AllReduce kick."""
                ps = psS.tile([128, 176], F32, tag="s")
                for glo, gn, _ in XCG:
                    s_matmuls(r, bt, ps, glo, gn)
                nc.vector.tensor_copy(sbb[:, bt * 160:(bt + 1) * 160],
                                      ps[:, :160])
                bounce_bt(bt)
                return ps

            def bounce_bt(bt):
                nc.sync.dma_start(
                    bounce_in[bt * 128:(bt + 1) * 128, :],
                    sbb[:, bt * 160:(bt + 1) * 160])
                if NO_CC:
                    nc.sync.dma_start(
                        bounce_out[bt * 128:(bt + 1) * 128, :],
                        bounce_in[bt * 128:(bt + 1) * 128, :])
                else:
                    nc.gpsimd.collective_compute(
                        "AllReduce", ADD,
                        ins=[bounce_in[bt * 128:(bt + 1) * 128, :].opt()],
                        outs=[bounce_out[bt * 128:(bt + 1) * 128, :].opt()],
                        replica_groups=[list(range(CORES))],
                    )
                # land the reduced s directly in the pad-32 transpose
                # staging layout (cols d*32 + i)
                nc.sync.dma_start(
                    s_bf[:, bt * 320:(bt + 1) * 320].rearrange(
                        "m (d i) -> m d i", d=D, i=32)[:, :, :16],
                    bounce_out[bt * 128:(bt + 1) * 128, :].rearrange(
                        "m (d i) -> m d i", d=D, i=16))

            def n2_partial(bt):
                ss = s_bf[:, bt * 320:(bt + 1) * 320].rearrange(
                    "m (d i) -> m d i", d=D, i=32)[:, :, :16]
                nc.vector.tensor_tensor(
                    sq[:, bt * 160:(bt + 1) * 160].rearrange(
                        "m (d i) -> m d i", d=D, i=16), ss, ss, MULT)
                nc.vector.tensor_reduce(
                    accb[:, bt:bt + 1],
                    sq[:, None, bt * 160:(bt + 1) * 160],
                    mybir.AxisListType.X, ADD)

            def alpha_final(psn):
                # alpha = n2 / ((n2+1)(sqrt(n2)+eps)); sqrt via ln/exp so
                # ACT stays on the exp/ln table set. psn: spare cols
                # [160:161] of the last s-contract PSUM tile.
                nc.vector.tensor_tensor(acc[:, :], accb[:, 0:1], accb[:, 1:2],
                                        ADD)
                nc.tensor.matmul(psn[:, 160:161], ones[:, :], acc[:, :],
                                 start=True, stop=True)
                nc.vector.tensor_copy(n2sb[:, :], psn[:, 160:161])
                nc.vector.tensor_scalar_add(t1[:, :], n2sb[:, :], 1.0)
                nc.vector.reciprocal(r1[:, :], t1[:, :])
                nc.scalar.activation(lnv[:, :], n2sb[:, :], AF.Ln)
                nc.scalar.activation(rt[:, :], lnv[:, :], AF.Exp, scale=0.5)
                nc.vector.tensor_tensor(alpha_bc[:, :], rt[:, :], r1[:, :],
                                        MULT)

            def vtd_path(bt):
                # s^T tiles for the wv matmul lhsT; the AllReduce-return
                # DMA already landed s in s_bf's pad-32 layout.
                for rnd, (dlo, nd) in enumerate(((0, 3), (3, 3), (6, 3),
                                                 (9, 1))):
                    ptv = psV.tile([128, 128], BF16, tag="vt")
                    nc.tensor.transpose(
                        ptv[:32 * nd, :],
                        s_bf[:, bt * 320 + dlo * 32:
                             bt * 320 + (dlo + nd) * 32],
                        ident[:, :])
                    nc.vector.tensor_copy(
                        vtd[:32 * nd, rnd * 256 + bt * 128:
                            rnd * 256 + bt * 128 + 128],
                        ptv[:32 * nd, :])

            def vt_ap(d, bt):
                return vtd[32 * (d % 3):32 * (d % 3) + 16,
                           (d // 3) * 256 + bt * 128:
                           (d // 3) * 256 + bt * 128 + 128]

            # per-digit g-phase mode: ACT drain + DVE mult, ACT drain +
            # Pool mult, or DVE mult reading PSUM f32 directly (1x)
            G_MODES = (["ad"] * (D - POOL_G_D - DVE_DIRECT_D)
                       + ["dv"] * DVE_DIRECT_D + ["ap"] * POOL_G_D)

            def g_phase(bt):
                # G[b, (d,p)] = sum_j x * (W @ s^T)   (raw, no alpha)
                for d in range(D):
                    mode = G_MODES[d]
                    eng = nc.gpsimd if mode == "ap" else nc.vector
                    yb = Y[:, (d % 3) * JP:(d % 3) * JP + JP]
                    ywb = Yw[:, (d % 3) * JP:(d % 3) * JP + JP]
                    for h in range(3):
                        pw = psW.tile([128, 384], F32, tag="wv")
                        q = 32 * (d % 3)
                        nc.tensor.matmul(
                            pw[:, :], vt_ap(d, bt),
                            wg[q:q + 16,
                               d * JP + h * 384:d * JP + (h + 1) * 384],
                            start=True, stop=True)
                        xs = xbp[:, bt * JP + h * 384:bt * JP + (h + 1) * 384]
                        if mode == "dv":
                            nc.vector.tensor_tensor(
                                yb[:, h * 384:(h + 1) * 384], pw[:, :], xs,
                                MULT)
                        else:
                            nc.scalar.copy(ywb[:, h * 384:(h + 1) * 384],
                                           pw[:, :])
                            eng.tensor_tensor(
                                yb[:, h * 384:(h + 1) * 384],
                                ywb[:, h * 384:(h + 1) * 384], xs, MULT)
                    eng.tensor_tensor(yb[:, 0:576], yb[:, 0:576],
                                      yb[:, 576:1152], ADD)
                    eng.tensor_tensor(yb[:, 0:288], yb[:, 0:288],
                                      yb[:, 288:576], ADD)
                    eng.tensor_tensor(
                        G[:, bt * 1440 + d * 144:bt * 1440 + (d + 1) * 144],
                        yb[:, 0:144], yb[:, 144:288], ADD)

            def e_update(r, bt):
                gb = G[:, bt * 1440:(bt + 1) * 1440]
                eb = E[:, bt * 1440:(bt + 1) * 1440]
                if r == 0:
                    nc.scalar.activation(eb, gb, AF.Exp, scale=alpha_bc[:, :1])
                else:
                    etb = Et[:, bt * 1440:(bt + 1) * 1440]
                    nc.scalar.activation(etb, gb, AF.Exp,
                                         scale=alpha_bc[:, :1])
                    nc.vector.tensor_tensor(eb, eb, etb, MULT)

            def softmax_c(bt):
                eb = E[:, bt * 1440:(bt + 1) * 1440]
                zb = zs[:, bt * 720:(bt + 1) * 720]
                nc.vector.tensor_tensor(zb[:, 0:720], eb[:, 0:720],
                                        eb[:, 720:1440], ADD)
                nc.vector.tensor_tensor(zb[:, 0:288], zb[:, 0:288],
                                        zb[:, 288:576], ADD)
                nc.vector.tensor_tensor(zb[:, 0:144], zb[:, 0:144],
                                        zb[:, 144:288], ADD)
                nc.vector.tensor_tensor(zb[:, 0:144], zb[:, 0:144],
                                        zb[:, 576:720], ADD)
                with nc.allow_low_precision(
                        reason="softmax denominators are O(1); bf16 "
                               "reciprocal error is below the bf16 c noise"):
                    nc.vector.reciprocal(zrecb[:, bt * 144:(bt + 1) * 144],
                                         zb[:, 0:144])
                nc.vector.tensor_tensor(
                    cb[:, bt * 1440:(bt + 1) * 1440].rearrange(
                        "m (d p) -> m d p", d=D, p=PL),
                    eb.rearrange("m (d p) -> m d p", d=D, p=PL),
                    zrecb[:, None, bt * 144:(bt + 1) * 144].to_broadcast(
                        (128, D, PL)),
                    MULT)

            # ---------------- schedule ----------------
            # Emission order == per-engine execution order. Rounds are
            # software-pipelined: the next round's bt0 softmax/xc/xbar
            # front is emitted between this round's two g-phases.
            for r in range(NROUT_RUN):
                if r == 0:
                    s_phase_front(0, 0)
                    s_phase_front(0, 1)
                s_phase_mm(r, 0)
                if r == 0:
                    load_rest()
                ps = s_phase_mm(r, 1)
                last = (r == NROUT_RUN - 1)
                if last:
                    n2_partial(0)
                    n2_partial(1)
                    alpha_final(ps)
                else:
                    vtd_path(0)
                    n2_partial(0)
                    vtd_path(1)
                    n2_partial(1)
                    alpha_final(ps)
                    g_phase(0)              # waits only on AllReduce(bt0)
                    e_update(r, 0)
                    s_phase_front(r + 1, 0)
                    g_phase(1)
                    e_update(r, 1)
                    s_phase_front(r + 1, 1)

            # ---- v = alpha * s ; layout already matches v_out ----
            for bt in range(2):
                nc.vector.tensor_scalar(
                    sv[:, bt * 160:(bt + 1) * 160].rearrange(
                        "m (d i) -> m d i", d=D, i=16),
                    s_bf[:, bt * 320:(bt + 1) * 320].rearrange(
                        "m (d i) -> m d i", d=D, i=32)[:, :, :16],
                    alpha_bc[:, :1], None, MULT)
                nc.sync.dma_start(v_d.ap()[bt * 128:(bt + 1) * 128, :, :],
                                  sv[:, bt * 160:(bt + 1) * 160])

    nc.compile()
    return nc


def prep_inputs(primary_caps: np.ndarray, W: np.ndarray):
    """Host-side shard + layout prep. Returns in_maps for the 8 cores."""
    x = np.asarray(primary_caps, dtype=np.float32)
    Wf = np.asarray(W, dtype=np.float32)
    bf = ml_dtypes.bfloat16
    ident = np.eye(128, dtype=np.float32).astype(bf)
    in_maps = []
    for kcore in range(CORES):
        pk = slice(kcore * PL, (kcore + 1) * PL)
        xk = x[:, pk, :]                       # [256, 144, 8]
        Wk = Wf[:, pk, :, :]                   # [10, 144, 16, 8]

        # x_bp [128, 2*1152]: row b%128, col (bt*1152 + j*144 + p)
        xbp = xk.transpose(0, 2, 1).reshape(B, JP)
        xbp = xbp.reshape(2, 128, JP).transpose(1, 0, 2).reshape(128, 2 * JP)

        # x_t9 [128, 9*256]: 0.1*x^T: row (j*144+p)%128, col k*256 + b
        xt9 = 0.1 * xk.transpose(2, 1, 0).reshape(JP, B)
        xt9 = xt9.reshape(NK, 128, B).transpose(1, 0, 2).reshape(128, NK * B)

        # w_sc [128, 10*9*16]: row (j*144+p)%128, col (d*9+k)*16 + i
        wsc = Wk.transpose(0, 3, 1, 2).reshape(D, JP, I)
        wsc = wsc.reshape(D, NK, 128, I).transpose(2, 0, 1, 3)
        wsc = wsc.reshape(128, D * NK * I)

        # w_g [128, D*J*PL]: rows 32q+i (4 replicas), col d*1152 + j*144 + p
        wg1 = Wk.transpose(2, 0, 3, 1).reshape(I, D * JP)
        wg = np.zeros((128, D * JP), dtype=np.float32)
        for q in range(3):
            wg[32 * q:32 * q + I] = wg1

        in_maps.append({
            "x_bp": xbp.astype(bf),
            "x_t9": xt9.astype(bf),
            "w_sc": wsc.astype(bf),
            "w_g": wg.astype(bf),
            "ident": ident,
        })
    return in_maps


_NC_CACHE = None


def get_program():
    global _NC_CACHE
    if _NC_CACHE is None:
        _NC_CACHE = build_program()
    return _NC_CACHE


def kernel(primary_caps: np.ndarray, W: np.ndarray) -> np.ndarray:
    nc = get_program()
    in_maps = prep_inputs(primary_caps, W)
    res = run_bass_kernel_spmd(nc, in_maps, core_ids=list(range(CORES)))
    return np.asarray(res.results[0]["v_out"], dtype=np.float32)


if __name__ == "__main__":
    rng = np.random.default_rng(0)
    x = rng.standard_normal((B, P, J), dtype=np.float32)
    W = rng.standard_normal((D, P, I, J), dtype=np.float32).astype(np.float32)
    out = kernel(x, W)
    print("out", out.shape, out.dtype, float(np.abs(out).mean()))
